# revision 16
# baseline (speedup 1.0000x reference)
"""Trainium2 Bass kernel for nn_BridgeBlock (encoder/decoder bridge block).

Sharding: 8 cores = (batch 0..3) x (query-half 0..1). Each core computes its
512 query rows of both the encoder and decoder paths. K/V are computed
locally for all 1024 rows of the core's batch; per-core inputs are column-
rotated so the core's own 512 rows always sit in columns [0:512) (keeps the
program identical across cores / pure SPMD).

Layout: feature-major ("FM") — activations stored transposed [features, rows]
in SBUF as 8 tiles of [128, cols]. Weights-stationary matmuls produce FM
outputs; an activation-stationary matmul produces row-major V for attention.
Scores are computed transposed [k, q]; the softmax denominator comes free via
a ones-column appended to V; normalization is deferred to after A@V (recip =
exp(-ln), same ACT table set as the softmax exp). LayerNorm gain/bias, the
1/sqrt(dk) scale, and the V-bias (via softmax rows summing to 1) are folded
into weights/biases on the host. Decoder mask: own-block [512,512] tile mask
plus a per-core scalar for the other half (causal => other half is all-0 or
all-1; all-ones masks also supported). All matmuls run in float32r
(tf32-like, ~1.5e-4), everything else fp32.

SBUF: long-lived tensors allocate on the right side-stack, transients on the
left; PSUM uses 8 fixed bank tags.
"""
import numpy as np
from contextlib import ExitStack

import concourse.bass as bass
import concourse.tile as tile
import concourse.mybir as mybir
from concourse import bacc
from concourse.bass_utils import run_bass_kernel_spmd

F32 = mybir.dt.float32
F32R = mybir.dt.float32r
AF = mybir.ActivationFunctionType
OP = mybir.AluOpType

D = 1024      # d_model
FF = 4096     # d_ff
H = 16        # heads
DK = 64       # head dim
BTL = 64      # adapter bottleneck
Q = 512       # own query rows per core
T = 1024      # total rows per batch
NB = D // 128   # 8 feature tiles
EPS = 1e-5

_COMPILED = None
MARKS = []


def _build():
    nc = bacc.Bacc("TRN2", target_bir_lowering=False, debug=False)

    def din(name, shape, dt=F32R):
        return nc.dram_tensor(name, shape, dt, kind="ExternalInput").ap()

    xeT_d = din("xeT", [D, T])
    xdT_d = din("xdT", [D, T])
    maskP_d = din("maskP", [2 * 128, 2 * Q], F32)  # paired own-block mask
    otherm_d = din("otherm", [128, 1], F32)    # other-half mask scalar
    wq_d = din("wq", [D, D])
    wk_d = din("wk", [D, D])
    wv_d = din("wv", [D, D])
    wo_d = din("wo", [D, D])
    w1_d = din("w1", [D, FF])
    w2_d = din("w2", [FF, D])
    bqkv_d = din("bqkv", [128, NB * 2], F32)   # folded q/k biases, [128,8] each
    bo_d = din("bo", [128, NB], F32)           # bv @ wo fold (per out-feature)
    b1_d = din("b1", [128, FF // 128], F32)
    b2_d = din("b2", [128, NB], F32)
    adn_d = [din(f"adn{i}", [D, BTL]) for i in range(4)]
    aup_d = [din(f"aup{i}", [BTL, D]) for i in range(4)]
    adb_d = [din(f"adb{i}", [BTL, 1], F32) for i in range(4)]
    aub_d = [din(f"aub{i}", [128, NB], F32) for i in range(4)]

    encoutT_d = nc.dram_tensor("encoutT", [D, Q], F32, kind="ExternalOutput").ap()
    decoutT_d = nc.dram_tensor("decoutT", [D, Q], F32, kind="ExternalOutput").ap()

    # inline constants
    e2 = np.zeros((2, 128), np.float32)
    e2[0, 0:DK] = 1.0
    e2[1, DK:128] = 1.0
    e2_d = nc.inline_tensor(e2, name="e2c").ap()
    ones_row_d = nc.inline_tensor(np.ones((1, 128), np.float32), name="ones_row_c").ap()
    invd_row_d = nc.inline_tensor(np.full((1, 128), 1.0 / D, np.float32),
                                  name="invd_row_c").ap()
    ones_col_d = nc.inline_tensor(np.ones((128, 1), np.float32), name="ones_col_c").ap()

    MARKS.clear()

    def mark(label):
        MARKS.append((int(nc.get_next_instruction_name()[2:]), label))

    with tile.TileContext(nc) as tc, ExitStack() as top:
        cp = top.enter_context(tc.tile_pool(name="consts", bufs=1, side="left"))
        pp = top.enter_context(tc.tile_pool(name="psum", bufs=1, space="PSUM"))
        wp = top.enter_context(tc.tile_pool(name="wchunks", bufs=1, side="left"))

        def pbank(i, p=128, f=512, name="ps"):
            return pp.tile([p, f], F32, name=name, tag=f"bank{i}")

        # ---- constants / biases ----
        e2_s = cp.tile([2, 128], F32R, name="e2_s")
        nc.sync.dma_start(out=e2_s[:], in_=e2_d[:].bitcast(F32R))
        ones_row = cp.tile([1, 128], F32R, name="ones_row")
        nc.sync.dma_start(out=ones_row[:], in_=ones_row_d[:].bitcast(F32R))
        invd_row = cp.tile([1, 128], F32R, name="invd_row")
        nc.sync.dma_start(out=invd_row[:], in_=invd_row_d[:].bitcast(F32R))
        ones_col = cp.tile([128, 1], F32R, name="ones_col")
        nc.sync.dma_start(out=ones_col[:], in_=ones_col_d[:].bitcast(F32R))
        bqkv_s = cp.tile([128, NB * 2], F32, name="bqkv_s")
        nc.sync.dma_start(out=bqkv_s[:], in_=bqkv_d[:])
        bo_s = cp.tile([128, NB], F32, name="bo_s")
        nc.sync.dma_start(out=bo_s[:], in_=bo_d[:])
        b1_s = cp.tile([128, FF // 128], F32, name="b1_s")
        nc.sync.dma_start(out=b1_s[:], in_=b1_d[:])
        b2_s = cp.tile([128, NB], F32, name="b2_s")
        nc.sync.dma_start(out=b2_s[:], in_=b2_d[:])
        otherm_s = cp.tile([128, 1], F32, name="otherm_s")
        nc.sync.dma_start(out=otherm_s[:], in_=otherm_d[:])
        eps_t = cp.tile([1, 1], F32, name="eps_t")
        nc.vector.memset(eps_t[:], EPS)
        adb_s, aub_s = [], []
        for i in range(4):
            a = cp.tile([BTL, 1], F32, name=f"adb_s{i}", tag=f"adb_s{i}")
            nc.sync.dma_start(out=a[:], in_=adb_d[i][:])
            adb_s.append(a)
            u = cp.tile([128, NB], F32, name=f"aub_s{i}", tag=f"aub_s{i}")
            nc.sync.dma_start(out=u[:], in_=aub_d[i][:])
            aub_s.append(u)

        # ---- helpers ----
        def warm(dep_ap):
            """Tiny dep-gated fp32 matmul to keep the PE HAM clock from idling."""
            psd = pp.tile([1, 64], F32, name="warm_ps", tag="bank3")
            nc.tensor.matmul(psd[:], ones_row[0:1, 0:1].bitcast(F32),
                             dep_ap[0:1, 0:64], start=True, stop=True)

        def ln_block(lp, x_tiles, out_sl_writer, sl):
            """One 512-col LN block: stats + normalize.

            out = (x - mean_bcast) * rstd_bcast; mean bcast is issued early so
            the subtract half of the apply overlaps the rstd scalar chain.
            Uses only Copy/Ln/Exp ACT funcs (one table set).
            """
            ps_s = pbank(0, p=1, name="ln_ps_s")
            ps_q = pbank(1, p=1, name="ln_ps_q")
            for i in range(NB):
                sq = lp.tile([128, 512], F32R, name="ln_sq", tag="ln_sq", bufs=2)
                nc.vector.tensor_tensor(sq[:], x_tiles[i][:, sl],
                                        x_tiles[i][:, sl], OP.mult)
                nc.tensor.matmul(ps_s[:], ones_col[:], x_tiles[i][:, sl],
                                 start=(i == 0), stop=(i == NB - 1))
                nc.tensor.matmul(ps_q[:], ones_col[:], sq[:],
                                 start=(i == 0), stop=(i == NB - 1))
            sumr = lp.tile([1, 512], F32R, name="ln_sumr", tag="ln_sumr")
            nc.scalar.copy(sumr[:], ps_s[:])
            ps_m = pbank(2, name="ln_bc_m")      # mean bcast, available early
            nc.tensor.matmul(ps_m[:], invd_row[:], sumr[:], start=True, stop=True)
            s2 = lp.tile([1, 512], F32, name="ln_s2", tag="ln_s2")
            nc.vector.tensor_tensor(s2[:], sumr[:], sumr[:], OP.mult)
            nc.vector.tensor_scalar(s2[:], s2[:], 1.0 / D, None, OP.mult)
            nc.vector.tensor_tensor(s2[:], ps_q[:], s2[:], OP.subtract)
            warm(s2[:])
            nc.scalar.activation(s2[:], s2[:], AF.Ln, scale=1.0 / D,
                                 bias=eps_t[0:1, 0:1])
            rstd = lp.tile([1, 512], F32R, name="ln_rstd", tag="ln_rstd")
            nc.scalar.activation(rstd[:], s2[:], AF.Exp, scale=-0.5)
            warm(s2[:])
            ps_r = pbank(3, name="ln_bc_r")
            nc.tensor.matmul(ps_r[:], ones_row[:], rstd[:], start=True, stop=True)
            for i in range(NB):
                tmp = lp.tile([128, 512], F32, name="ln_tmp", tag="ln_tmp", bufs=3)
                nc.vector.tensor_tensor(tmp[:], x_tiles[i][:, sl], ps_m[:],
                                        OP.subtract)
                out_sl_writer(i, tmp, ps_r)

        def layernorm(pool, x_tiles, nm, cols):
            """Full FM layernorm into fresh [128, cols] f32r tiles."""
            out = [pool.tile([128, cols], F32R, name=f"{nm}{i}", tag=f"{nm}{i}")
                   for i in range(NB)]
            with tc.tile_pool(name=f"{nm}_scr", bufs=1, side="left") as lp:
                for blk in range(cols // 512):
                    sl = slice(512 * blk, 512 * blk + 512)

                    def wr(i, tmp, ps_r, sl=sl):
                        nc.vector.tensor_tensor(out[i][:, sl], tmp[:], ps_r[:],
                                                OP.mult)
                    ln_block(lp, x_tiles, wr, sl)
            return out

        def gemm_ws(w_dram, Kf, Mf, rhs_tiles, cols, evict, nm, rhs_off=0):
            """out[m, col] = sum_k w[k, m] * rhs[k, col]; evict(mi, blk, psum)."""
            nkt = Kf // 128
            nblk = cols // 512
            mpg = max(1, 4 // nblk)   # m-tiles per group (<=4 psum banks)
            ntiles_m = Mf // 128
            for mg in range((ntiles_m + mpg - 1) // mpg):
                m4n = min(mpg, ntiles_m - mpg * mg)
                psums = [[pbank(m4 * nblk + blk, name=f"{nm}_ps")
                          for blk in range(nblk)] for m4 in range(m4n)]
                cstart = 128 * mpg * mg
                cw = 128 * m4n
                for k in range(nkt):
                    ch = wp.tile([128, 512], F32R, name=f"{nm}_ch", tag="wchunk",
                                 bufs=3)
                    nc.sync.dma_start(
                        out=ch[:, 0:cw],
                        in_=w_dram[128 * k:128 * k + 128, cstart:cstart + cw])
                    for blk in range(nblk):
                        sl = slice(rhs_off + 512 * blk, rhs_off + 512 * blk + 512)
                        for m4 in range(m4n):
                            nc.tensor.matmul(
                                psums[m4][blk][:], ch[:, 128 * m4:128 * m4 + 128],
                                rhs_tiles[k][:, sl],
                                start=(k == 0), stop=(k == nkt - 1))
                for m4 in range(m4n):
                    for blk in range(nblk):
                        evict(mpg * mg + m4, blk, psums[m4][blk])

        def adapter(out_pool, x_tiles, ai, cols, extra_res, nm):
            """out = (extra_res or 0) + x + up(gelu(dn(LN(x)))). FM tiles out.

            LN runs block-wise: only one 512-col block of normalized data is
            alive at a time.
            """
            with ExitStack() as ies:
                ip = ies.enter_context(
                    tc.tile_pool(name=f"{nm}_scr", bufs=1, side="left"))
                nblk = cols // 512
                mT = ip.tile([BTL, cols], F32R, name=f"{nm}m", tag=f"{nm}m")
                dn_sb = []
                for k in range(NB):
                    w = ip.tile([128, BTL], F32R, name=f"{nm}dn{k}",
                                tag=f"{nm}dn{k}")
                    nc.sync.dma_start(out=w[:],
                                      in_=adn_d[ai][128 * k:128 * k + 128, :])
                    dn_sb.append(w)
                for blk in range(nblk):
                    sl = slice(512 * blk, 512 * blk + 512)
                    ps = pbank(blk % 2, p=BTL, name=f"{nm}_dps")

                    def wr(i, tmp, ps_r, ps=ps):
                        a = ip.tile([128, 512], F32R, name="ad_a", tag="ad_a",
                                    bufs=3)
                        nc.vector.tensor_tensor(a[:], tmp[:], ps_r[:],
                                                OP.mult)
                        nc.tensor.matmul(ps[:], dn_sb[i][:], a[:],
                                         start=(i == 0), stop=(i == NB - 1))
                    ln_block(ip, x_tiles, wr, sl)
                    nc.scalar.activation(mT[:, sl], ps[:], AF.Gelu,
                                         bias=adb_s[ai][:])
                up_sb = ip.tile([BTL, D], F32R, name=f"{nm}up", tag=f"{nm}up")
                nc.sync.dma_start(out=up_sb[:], in_=aup_d[ai][:])
                out = [out_pool.tile([128, cols], F32R, name=f"{nm}o{i}",
                                     tag=f"{nm}o{i}") for i in range(NB)]
                for m in range(NB):
                    for blk in range(nblk):
                        sl = slice(512 * blk, 512 * blk + 512)
                        ps = pbank(2 + (m * nblk + blk) % 2, name=f"{nm}_ups")
                        nc.tensor.matmul(ps[:], up_sb[:, 128 * m:128 * m + 128],
                                         mT[:, sl], start=True, stop=True)
                        tu = ip.tile([128, 512], F32, name="uptmp", tag="uptmp",
                                     bufs=2)
                        nc.scalar.activation(tu[:], ps[:], AF.Identity,
                                             bias=aub_s[ai][:, m:m + 1])
                        if extra_res is not None:
                            tu2 = ip.tile([128, 512], F32, name="uptmp2",
                                          tag="uptmp2", bufs=2)
                            nc.vector.tensor_tensor(tu2[:], tu[:],
                                                    x_tiles[m][:, sl], OP.add)
                            nc.vector.tensor_tensor(out[m][:, sl], tu2[:],
                                                    extra_res[m][:, sl], OP.add)
                        else:
                            nc.vector.tensor_tensor(out[m][:, sl], tu[:],
                                                    x_tiles[m][:, sl], OP.add)
            return out

        # ============================ block body ============================
        def half_block(xT_d_, is_dec):
            tagp = "d" if is_dec else "e"
            with ExitStack() as es:
                # right stack (long-lived): xown, proj
                xop = es.enter_context(
                    tc.tile_pool(name=f"{tagp}_xo", bufs=1, side="right"))
                xown = [xop.tile([128, Q], F32R, name=f"{tagp}xo{i}",
                                 tag=f"{tagp}xo{i}") for i in range(NB)]
                wop = es.enter_context(
                    tc.tile_pool(name=f"{tagp}_wo", bufs=1, side="right"))
                proj = [wop.tile([128, Q], F32R, name=f"{tagp}proj{i}",
                                 tag=f"{tagp}proj{i}") for i in range(NB)]

                # left stack: h1 outlives x/dad
                with ExitStack() as h1_es:
                    h1p = h1_es.enter_context(
                        tc.tile_pool(name=f"{tagp}_h1", bufs=1, side="left"))
                    if is_dec:
                        dad_es = ExitStack()
                        dadp = dad_es.enter_context(
                            tc.tile_pool(name="dadp", bufs=1, side="left"))
                    x_es = ExitStack()
                    xp = x_es.enter_context(
                        tc.tile_pool(name=f"{tagp}_x", bufs=1, side="left"))
                    xT = []
                    for i in range(NB):
                        t = xp.tile([128, T], F32R, name=f"{tagp}xT{i}",
                                    tag=f"{tagp}xT{i}")
                        nc.sync.dma_start(out=t[:],
                                          in_=xT_d_[128 * i:128 * i + 128, :])
                        xT.append(t)
                    for i in range(NB):
                        nc.vector.tensor_copy(xown[i][:], xT[i][:, 0:Q])
                    if is_dec:
                        mark(f"{tagp}:ad1")
                        dad = adapter(dadp, xT, 0, T, None, "ddad")
                        x_es.close()
                        mark(f"{tagp}:ln1")
                        h1 = layernorm(h1p, dad, f"{tagp}h1", T)
                        dad_es.close()
                    else:
                        mark(f"{tagp}:ln1")
                        h1 = layernorm(h1p, xT, f"{tagp}h1", T)
                        x_es.close()

                    # open attention-lifetime pools late (right side)
                    at_es = ExitStack()
                    atp = at_es.enter_context(
                        tc.tile_pool(name=f"{tagp}_attn", bufs=1, side="right"))
                    attnT = [atp.tile([128, Q], F32R, name=f"{tagp}atn{i}",
                                      tag=f"{tagp}atn{i}") for i in range(NB)]
                    if is_dec:
                        maskP = []
                        for i in range(2):
                            m = atp.tile([128, 2 * Q], F32, name=f"maskP{i}",
                                         tag=f"maskP{i}")
                            nc.sync.dma_start(
                                out=m[:], in_=maskP_d[128 * i:128 * i + 128, :])
                            maskP.append(m)
                    qk_es = ExitStack()
                    qkp = qk_es.enter_context(
                        tc.tile_pool(name=f"{tagp}_qk", bufs=1, side="right"))
                    V = [qkp.tile([128, H * (DK + 1)], F32R, name=f"{tagp}V{i}",
                                  tag=f"{tagp}V{i}") for i in range(NB)]
                    QT = [qkp.tile([128, Q], F32R, name=f"{tagp}QT{i}",
                                   tag=f"{tagp}QT{i}") for i in range(NB)]
                    KT = [qkp.tile([128, T], F32R, name=f"{tagp}KT{i}",
                                   tag=f"{tagp}KT{i}") for i in range(NB)]

                    # --- V (activation-stationary, wv streamed per half) ---
                    mark(f"{tagp}:V")
                    with tc.tile_pool(name=f"{tagp}_wv", bufs=1,
                                      side="left") as wvp:
                        for nblk in range(2):
                            wv_sb = []
                            for k in range(NB):
                                w = wvp.tile([128, 512], F32R,
                                             name=f"{tagp}wv{k}",
                                             tag=f"{tagp}wv{k}", bufs=1)
                                nc.sync.dma_start(
                                    out=w[:],
                                    in_=wv_d[128 * k:128 * k + 128,
                                             512 * nblk:512 * nblk + 512])
                                wv_sb.append(w)
                            for mr in range(NB):
                                ps = pbank((mr % 2) * 2 + nblk,
                                           name=f"{tagp}v_ps")
                                for k in range(NB):
                                    nc.tensor.matmul(
                                        ps[:], h1[k][:, 128 * mr:128 * mr + 128],
                                        wv_sb[k][:],
                                        start=(k == 0), stop=(k == NB - 1))
                                dst = V[mr][:].rearrange("p (h e) -> p h e", h=H)
                                nc.vector.tensor_copy(
                                    dst[:, 8 * nblk:8 * nblk + 8, 0:DK],
                                    ps[:].rearrange("p (h e) -> p h e",
                                                    h=8, e=DK))
                        for mr in range(NB):
                            onesv = V[mr][:].rearrange(
                                "p (h e) -> p h e", h=H)[:, :, DK:DK + 1]
                            nc.vector.tensor_copy(
                                onesv[:], ones_col[:].to_broadcast((128, H, 1)))

                    # --- Q, K ---
                    mark(f"{tagp}:Q")

                    def ev_q(mi, blk, ps):
                        nc.scalar.activation(QT[mi][:], ps[:], AF.Identity,
                                             bias=bqkv_s[:, mi:mi + 1])
                    gemm_ws(wq_d, D, D, h1, 512, ev_q, f"{tagp}q")
                    mark(f"{tagp}:K")

                    def ev_k(mi, blk, ps):
                        nc.scalar.activation(
                            KT[mi][:, 512 * blk:512 * blk + 512], ps[:],
                            AF.Identity, bias=bqkv_s[:, NB + mi:NB + mi + 1])
                    gemm_ws(wk_d, D, D, h1, T, ev_k, f"{tagp}k")

                # --- attention (h1 freed) ---
                mark(f"{tagp}:attn")
                with tc.tile_pool(name=f"{tagp}_ascr", bufs=1, side="left") as ascp:
                    for tpair in range(NB):
                        attnP = ascp.tile([128, Q], F32, name="attnP",
                                          tag="attnP", bufs=2)
                        pden = ascp.tile([2, Q], F32, name="pden", tag="pden",
                                         bufs=2)
                        # AV psum: both heads side by side (2 banks)
                        ps_av = pp.tile([DK + 1, 2 * Q], F32, name=f"{tagp}av_ps",
                                        tag="av2")
                        for hh in range(2):
                            h = 2 * tpair + hh
                            s = DK * hh
                            for ktp in range(NB // 2):
                                # scores for two k-tiles into one 2-bank psum
                                ps_sc = pp.tile([128, 2 * Q], F32,
                                                name=f"{tagp}sc_ps", tag="sc2")
                                for u in range(2):
                                    kt = 2 * ktp + u
                                    nc.tensor.matmul(
                                        ps_sc[:, Q * u:Q * u + Q],
                                        KT[tpair][s:s + DK,
                                                  128 * kt:128 * kt + 128],
                                        QT[tpair][s:s + DK, :],
                                        start=True, stop=True)
                                em = ascp.tile([128, 2 * Q], F32R, name="expm",
                                               tag="expm", bufs=3)
                                if is_dec and ktp < 2:
                                    ex = ascp.tile([128, 2 * Q], F32, name="exps",
                                                   tag="exps", bufs=2)
                                    nc.scalar.activation(ex[:], ps_sc[:], AF.Exp)
                                    nc.vector.tensor_tensor(em[:], ex[:],
                                                            maskP[ktp][:],
                                                            OP.mult)
                                elif is_dec:
                                    ex = ascp.tile([128, 2 * Q], F32, name="exps",
                                                   tag="exps", bufs=2)
                                    nc.scalar.activation(ex[:], ps_sc[:], AF.Exp)
                                    nc.vector.tensor_scalar(
                                        em[:], ex[:], otherm_s[:, 0:1], None,
                                        OP.mult)
                                else:
                                    nc.scalar.activation(em[:], ps_sc[:], AF.Exp)
                                for u in range(2):
                                    kt = 2 * ktp + u
                                    nc.tensor.matmul(
                                        ps_av[:, Q * hh:Q * hh + Q],
                                        V[kt][:, 65 * h:65 * h + 65],
                                        em[:, Q * u:Q * u + Q],
                                        start=(kt == 0), stop=(kt == NB - 1))
                        stage = ascp.tile([DK + 1, 2 * Q], F32, name="avstage",
                                          tag="avstage", bufs=2)
                        nc.scalar.copy(stage[:], ps_av[:])
                        for hh in range(2):
                            s = DK * hh
                            nc.sync.dma_start(out=attnP[s:s + DK, :],
                                              in_=stage[0:DK, Q * hh:Q * hh + Q])
                            nc.sync.dma_start(
                                out=pden[hh:hh + 1, :],
                                in_=stage[DK:DK + 1, Q * hh:Q * hh + Q])
                        plnd = ascp.tile([2, Q], F32, name="plnd", tag="plnd")
                        nc.scalar.activation(plnd[:], pden[:], AF.Ln)
                        warm(plnd[:])
                        prec = ascp.tile([2, Q], F32R, name="prec", tag="prec")
                        nc.scalar.activation(prec[:], plnd[:], AF.Exp, scale=-1.0)
                        warm(plnd[:])
                        ps_b = pp.tile([128, Q], F32, name=f"{tagp}bc_ps",
                                       tag="av2")
                        nc.tensor.matmul(ps_b[:], e2_s[:], prec[:],
                                         start=True, stop=True)
                        nc.vector.tensor_tensor(attnT[tpair][:], attnP[:],
                                                ps_b[:], OP.mult)
                qk_es.close()

                # --- Wo projection (+ residual & bo bias) ---
                if is_dec:
                    def ev_o(mi, blk, ps):
                        nc.scalar.activation(proj[mi][:], ps[:], AF.Identity,
                                             bias=bo_s[:, mi:mi + 1])
                else:
                    def ev_o(mi, blk, ps):
                        tz = wp.tile([128, 512], F32, name="wotmp", tag="wotmp",
                                     bufs=2)
                        nc.scalar.activation(tz[:], ps[:], AF.Identity,
                                             bias=bo_s[:, mi:mi + 1])
                        nc.vector.tensor_tensor(proj[mi][:], tz[:], xown[mi][:],
                                                OP.add)
                mark(f"{tagp}:wo")
                gemm_ws(wo_d, D, D, attnT, 512, ev_o, f"{tagp}o")
                at_es.close()

                # --- post-attention / FFN ---
                with ExitStack() as post_es:
                    if is_dec:
                        mark(f"{tagp}:ad2")
                        pop = post_es.enter_context(
                            tc.tile_pool(name=f"{tagp}_post", bufs=1,
                                         side="right"))
                        r1 = adapter(pop, proj, 1, Q, xown, "dr1")
                        ffn_res = r1
                    else:
                        ffn_res = proj

                    ffp = post_es.enter_context(
                        tc.tile_pool(name=f"{tagp}_ff", bufs=1, side="right"))
                    ffo = [ffp.tile([128, Q], F32R, name=f"{tagp}ffo{i}",
                                    tag=f"{tagp}ffo{i}") for i in range(NB)]

                    h2_es = ExitStack()
                    h2p = h2_es.enter_context(
                        tc.tile_pool(name=f"{tagp}_h2", bufs=1, side="left"))
                    if is_dec:
                        mark(f"{tagp}:ad3")
                        d2_es = ExitStack()
                        d2p = d2_es.enter_context(
                            tc.tile_pool(name="d2p", bufs=1, side="left"))
                        d2 = adapter(d2p, r1, 2, Q, None, "dd2")
                        mark(f"{tagp}:ln2")
                        h2 = layernorm(h2p, d2, f"{tagp}h2", Q)
                        d2_es.close()
                    else:
                        mark(f"{tagp}:ln2")
                        h2 = layernorm(h2p, ffn_res, f"{tagp}h2", Q)

                    yt_es = ExitStack()
                    ytp = yt_es.enter_context(
                        tc.tile_pool(name=f"{tagp}_yt", bufs=1, side="right"))
                    yT = [ytp.tile([128, Q], F32R, name=f"{tagp}yT{i}",
                                   tag=f"{tagp}yT{i}") for i in range(FF // 128)]

                    mark(f"{tagp}:w1")

                    def ev_y(mi, blk, ps):
                        nc.scalar.activation(yT[mi][:], ps[:], AF.Gelu,
                                             bias=b1_s[:, mi:mi + 1])
                    gemm_ws(w1_d, D, FF, h2, 512, ev_y, f"{tagp}w1")
                    mark(f"{tagp}:w2")
                    h2_es.close()

                    if is_dec:
                        def ev_z(mi, blk, ps):
                            nc.scalar.activation(ffo[mi][:], ps[:], AF.Identity,
                                                 bias=b2_s[:, mi:mi + 1])
                    else:
                        def ev_z(mi, blk, ps):
                            tz = wp.tile([128, 512], F32, name="ztmp",
                                         tag="wotmp", bufs=2)
                            nc.scalar.activation(tz[:], ps[:], AF.Identity,
                                                 bias=b2_s[:, mi:mi + 1])
                            nc.vector.tensor_tensor(ffo[mi][:], tz[:],
                                                    ffn_res[mi][:], OP.add)
                    gemm_ws(w2_d, FF, D, yT, 512, ev_z, f"{tagp}w2")
                    yt_es.close()

                    if is_dec:
                        mark(f"{tagp}:ad4")
                        doutp = post_es.enter_context(
                            tc.tile_pool(name=f"{tagp}_dout", bufs=1,
                                         side="right"))
                        dout = adapter(doutp, ffo, 3, Q, ffn_res, "ddo")
                        for i in range(NB):
                            nc.sync.dma_start(
                                out=decoutT_d[128 * i:128 * i + 128, :],
                                in_=dout[i][:].bitcast(F32))
                    else:
                        for i in range(NB):
                            nc.sync.dma_start(
                                out=encoutT_d[128 * i:128 * i + 128, :],
                                in_=ffo[i][:].bitcast(F32))

        half_block(xeT_d, is_dec=False)
        half_block(xdT_d, is_dec=True)

    nc.compile()
    return nc


def _get_program():
    global _COMPILED
    if _COMPILED is None:
        _COMPILED = _build()
    return _COMPILED


# ----------------------------------------------------------------------------
# host wrapper
# ----------------------------------------------------------------------------

def _np(x):
    return np.asarray(x, dtype=np.float32)


def make_in_maps(enc_x, dec_x, params, dec_causal_mask):
    p = params
    enc_x = _np(enc_x)
    dec_x = _np(dec_x)
    mask01 = np.asarray(dec_causal_mask)[0, 0].astype(np.float32)  # [T, T]

    def fold(g, b, w):
        w = _np(w)
        return (_np(g)[:, None] * w), (_np(b) @ w)

    wqkv = _np(p['enc_attn']['w_qkv'])
    g1, b1g = _np(p['enc_ln1_g']), _np(p['enc_ln1_b'])
    wq, bq = fold(g1, b1g, wqkv[:, 0:D])
    wq, bq = wq / 8.0, bq / 8.0
    wk, bk = fold(g1, b1g, wqkv[:, D:2 * D])
    wv, bv = fold(g1, b1g, wqkv[:, 2 * D:3 * D])
    wo = _np(p['enc_attn']['w_o'])
    bo = bv @ wo                      # v-bias folded through softmax (rows sum 1)
    g2, b2g = _np(p['enc_ln2_g']), _np(p['enc_ln2_b'])
    w1, b1f = fold(g2, b2g, _np(p['enc_ff']['w1']))
    b1f = b1f + _np(p['enc_ff']['b1'])
    w2 = _np(p['enc_ff']['w2'])
    b2f = _np(p['enc_ff']['b2'])

    def bias_cols(b, f):
        return np.ascontiguousarray(b.reshape(f // 128, 128).T).astype(np.float32)

    shared = {
        'wq': np.ascontiguousarray(wq), 'wk': np.ascontiguousarray(wk),
        'wv': np.ascontiguousarray(wv), 'wo': wo,
        'w1': np.ascontiguousarray(w1), 'w2': w2,
        'bqkv': np.concatenate([bias_cols(bq, D), bias_cols(bk, D)], axis=1),
        'bo': bias_cols(bo, D),
        'b1': bias_cols(b1f, FF), 'b2': bias_cols(b2f, D),
    }
    for i, nmi in enumerate(['adapt_pre_attn', 'adapt_post_attn',
                             'adapt_pre_ff', 'adapt_post_ff']):
        a = p[nmi]
        dw, db = fold(_np(a['ln_g']), _np(a['ln_b']), _np(a['down_w']))
        db = db + _np(a['down_b'])
        shared[f'adn{i}'] = np.ascontiguousarray(dw)
        shared[f'adb{i}'] = db.reshape(BTL, 1).astype(np.float32)
        shared[f'aup{i}'] = _np(a['up_w'])
        shared[f'aub{i}'] = bias_cols(_np(a['up_b']), D)

    in_maps = []
    for c in range(8):
        b, h = c // 2, c % 2
        own = np.arange(512 * h, 512 * h + 512)
        oth = np.arange(512 * (1 - h), 512 * (1 - h) + 512)
        perm = np.concatenate([own, oth])
        im = dict(shared)
        im['xeT'] = np.ascontiguousarray(enc_x[b][perm].T)     # [D, T] own-first
        im['xdT'] = np.ascontiguousarray(dec_x[b][perm].T)
        mo = mask01[own][:, own].T                                  # [k, q] own
        im['maskP'] = np.ascontiguousarray(np.concatenate(
            [np.concatenate([mo[256 * j:256 * j + 128, :],
                             mo[256 * j + 128:256 * j + 256, :]], axis=1)
             for j in range(2)], axis=0))
        other_blk = mask01[own][:, oth]
        ov = float(other_blk.flat[0])
        if not np.all(other_blk == ov):
            raise ValueError("decoder mask other-half is not constant per core; "
                             "unsupported mask structure")
        im['otherm'] = np.full((128, 1), ov, np.float32)
        in_maps.append(im)
    return in_maps


def assemble(results, B):
    enc_out = np.empty((B, T, D), np.float32)
    dec_out = np.empty((B, T, D), np.float32)
    for c in range(8):
        b, h = c // 2, c % 2
        own = slice(512 * h, 512 * h + 512)
        enc_out[b, own] = results[c]['encoutT'].T
        dec_out[b, own] = results[c]['decoutT'].T
    return enc_out, dec_out


def kernel(enc_x, dec_x, params, dec_causal_mask, cross_mask):
    in_maps = make_in_maps(enc_x, dec_x, params, dec_causal_mask)
    nc = _get_program()
    res = run_bass_kernel_spmd(nc, in_maps, core_ids=list(range(8)))
    return assemble(res.results, np.asarray(enc_x).shape[0])


# revision 17
# speedup vs baseline: 1.3719x; 1.3719x over previous
"""Trainium2 Bass kernel for nn_BridgeBlock (encoder/decoder bridge block).

Sharding: 8 cores = (batch 0..3) x (query-half 0..1). Each core computes its
512 query rows of both the encoder and decoder paths. K/V are computed
locally for all 1024 rows of the core's batch; per-core inputs are column-
rotated so the core's own 512 rows always sit in columns [0:512) (keeps the
program identical across cores / pure SPMD).

Layout: feature-major ("FM") — activations stored transposed [features, rows]
in SBUF as 8 tiles of [128, cols]. Weights-stationary matmuls produce FM
outputs; an activation-stationary matmul produces row-major V for attention.
Scores are computed transposed [k, q]; the softmax denominator comes free via
a ones-column appended to V; normalization is deferred to after A@V (recip =
exp(-ln), same ACT table set as the softmax exp). LayerNorm gain/bias, the
1/sqrt(dk) scale, and the V-bias (via softmax rows summing to 1) are folded
into weights/biases on the host. Decoder mask: own-block [512,512] tile mask
plus a per-core scalar for the other half (causal => other half is all-0 or
all-1; all-ones masks also supported). All matmuls run in float32r
(tf32-like, ~1.5e-4), everything else fp32.

SBUF: long-lived tensors allocate on the right side-stack, transients on the
left; PSUM uses 8 fixed bank tags.
"""
import numpy as np
from contextlib import ExitStack

import concourse.bass as bass
import concourse.tile as tile
import concourse.mybir as mybir
from concourse import bacc
from concourse.bass_utils import run_bass_kernel_spmd

F32 = mybir.dt.float32
F32R = mybir.dt.float32r
AF = mybir.ActivationFunctionType
OP = mybir.AluOpType

D = 1024      # d_model
FF = 4096     # d_ff
H = 16        # heads
DK = 64       # head dim
BTL = 64      # adapter bottleneck
Q = 512       # own query rows per core
T = 1024      # total rows per batch
NB = D // 128   # 8 feature tiles
EPS = 1e-5

_COMPILED = None
MARKS = []


def _build():
    nc = bacc.Bacc("TRN2", target_bir_lowering=False, debug=False)

    def din(name, shape, dt=F32R):
        return nc.dram_tensor(name, shape, dt, kind="ExternalInput").ap()

    xeT_d = din("xeT", [D, T])
    xdT_d = din("xdT", [D, T])
    maskP_d = din("maskP", [2 * 128, 2 * Q], F32)  # paired own-block mask
    otherm_d = din("otherm", [128, 1], F32)    # other-half mask scalar
    wq_d = din("wq", [D, D])
    wk_d = din("wk", [D, D])
    wv_d = din("wv", [D, D])
    wo_d = din("wo", [D, D])
    w1_d = din("w1", [D, FF])
    w2_d = din("w2", [FF, D])
    bqkv_d = din("bqkv", [128, NB * 2], F32)   # folded q/k biases, [128,8] each
    bo_d = din("bo", [128, NB], F32)           # bv @ wo fold (per out-feature)
    b1_d = din("b1", [128, FF // 128], F32)
    b2_d = din("b2", [128, NB], F32)
    adn_d = [din(f"adn{i}", [D, BTL]) for i in range(4)]
    aup_d = [din(f"aup{i}", [BTL, D]) for i in range(4)]
    adb_d = [din(f"adb{i}", [BTL, 1], F32) for i in range(4)]
    aub_d = [din(f"aub{i}", [128, NB], F32) for i in range(4)]

    encoutT_d = nc.dram_tensor("encoutT", [D, Q], F32, kind="ExternalOutput").ap()
    decoutT_d = nc.dram_tensor("decoutT", [D, Q], F32, kind="ExternalOutput").ap()

    # inline constants
    e2 = np.zeros((2, 128), np.float32)
    e2[0, 0:DK] = 1.0
    e2[1, DK:128] = 1.0
    e2_d = nc.inline_tensor(e2, name="e2c").ap()
    ones_row_d = nc.inline_tensor(np.ones((1, 128), np.float32), name="ones_row_c").ap()
    invd_row_d = nc.inline_tensor(np.full((1, 128), 1.0 / D, np.float32),
                                  name="invd_row_c").ap()
    ones_col_d = nc.inline_tensor(np.ones((128, 1), np.float32), name="ones_col_c").ap()

    MARKS.clear()

    def mark(label):
        MARKS.append((int(nc.get_next_instruction_name()[2:]), label))

    with tile.TileContext(nc) as tc, ExitStack() as top:
        cp = top.enter_context(tc.tile_pool(name="consts", bufs=1, side="left"))
        pp = top.enter_context(tc.tile_pool(name="psum", bufs=1, space="PSUM"))
        wp = top.enter_context(tc.tile_pool(name="wchunks", bufs=1, side="left"))

        def ptile(tag, p=128, f=1024, name="ps"):
            return pp.tile([p, f], F32, name=name, tag=tag)

        # ---- constants / biases ----
        e2_s = cp.tile([2, 128], F32R, name="e2_s")
        nc.sync.dma_start(out=e2_s[:], in_=e2_d[:].bitcast(F32R))
        ones_row = cp.tile([1, 128], F32R, name="ones_row")
        nc.sync.dma_start(out=ones_row[:], in_=ones_row_d[:].bitcast(F32R))
        invd_row = cp.tile([1, 128], F32R, name="invd_row")
        nc.sync.dma_start(out=invd_row[:], in_=invd_row_d[:].bitcast(F32R))
        ones_col = cp.tile([128, 1], F32R, name="ones_col")
        nc.sync.dma_start(out=ones_col[:], in_=ones_col_d[:].bitcast(F32R))
        bqkv_s = cp.tile([128, NB * 2], F32, name="bqkv_s")
        nc.sync.dma_start(out=bqkv_s[:], in_=bqkv_d[:])
        bo_s = cp.tile([128, NB], F32, name="bo_s")
        nc.sync.dma_start(out=bo_s[:], in_=bo_d[:])
        b1_s = cp.tile([128, FF // 128], F32, name="b1_s")
        nc.sync.dma_start(out=b1_s[:], in_=b1_d[:])
        b2_s = cp.tile([128, NB], F32, name="b2_s")
        nc.sync.dma_start(out=b2_s[:], in_=b2_d[:])
        otherm_s = cp.tile([128, 1], F32, name="otherm_s")
        nc.sync.dma_start(out=otherm_s[:], in_=otherm_d[:])
        eps_t = cp.tile([1, 1], F32, name="eps_t")
        nc.vector.memset(eps_t[:], EPS)
        adb_s, aub_s = [], []
        for i in range(4):
            a = cp.tile([BTL, 1], F32, name=f"adb_s{i}", tag=f"adb_s{i}")
            nc.sync.dma_start(out=a[:], in_=adb_d[i][:])
            adb_s.append(a)
            u = cp.tile([128, NB], F32, name=f"aub_s{i}", tag=f"aub_s{i}")
            nc.sync.dma_start(out=u[:], in_=aub_d[i][:])
            aub_s.append(u)

        # ---- helpers ----
        def warm(dep_ap):
            """Tiny dep-gated fp32 matmul to keep the PE HAM clock from idling."""
            psd = pp.tile([1, 64], F32, name="warm_ps", tag="P2")
            nc.tensor.matmul(psd[:], ones_row[0:1, 0:1].bitcast(F32),
                             dep_ap[0:1, 0:64], start=True, stop=True)

        def ln_block(lp, x_tiles, out_sl_writer, sl):
            """One 512-col LN block: stats + normalize.

            out = (x - mean_bcast) * rstd_bcast; mean bcast is issued early so
            the subtract half of the apply overlaps the rstd scalar chain.
            Uses only Copy/Ln/Exp ACT funcs (one table set).
            """
            st = ptile("P2", p=1, name="ln_stats")
            ps_s = st[0:1, 0:512]
            ps_q = st[0:1, 512:1024]
            for i in range(NB):
                sq = lp.tile([128, 512], F32R, name="ln_sq", tag="ln_sq", bufs=2)
                nc.vector.tensor_tensor(sq[:], x_tiles[i][:, sl],
                                        x_tiles[i][:, sl], OP.mult)
                nc.tensor.matmul(ps_s[:], ones_col[:], x_tiles[i][:, sl],
                                 start=(i == 0), stop=(i == NB - 1))
                nc.tensor.matmul(ps_q[:], ones_col[:], sq[:],
                                 start=(i == 0), stop=(i == NB - 1))
            sumr = lp.tile([1, 512], F32R, name="ln_sumr", tag="ln_sumr")
            nc.scalar.copy(sumr[:], ps_s[:])
            bc = ptile("P3", name="ln_bc")
            ps_m = bc[:, 0:512]                  # mean bcast, available early
            nc.tensor.matmul(ps_m[:], invd_row[:], sumr[:], start=True, stop=True)
            s2 = lp.tile([1, 512], F32, name="ln_s2", tag="ln_s2")
            nc.vector.tensor_tensor(s2[:], sumr[:], sumr[:], OP.mult)
            nc.vector.tensor_scalar(s2[:], s2[:], 1.0 / D, None, OP.mult)
            nc.vector.tensor_tensor(s2[:], ps_q[:], s2[:], OP.subtract)
            warm(s2[:])
            nc.scalar.activation(s2[:], s2[:], AF.Ln, scale=1.0 / D,
                                 bias=eps_t[0:1, 0:1])
            rstd = lp.tile([1, 512], F32R, name="ln_rstd", tag="ln_rstd")
            nc.scalar.activation(rstd[:], s2[:], AF.Exp, scale=-0.5)
            ps_r = bc[:, 512:1024]
            nc.tensor.matmul(ps_r[:], ones_row[:], rstd[:], start=True, stop=True)
            for i in range(NB):
                tmp = lp.tile([128, 512], F32, name="ln_tmp", tag="ln_tmp", bufs=3)
                nc.vector.tensor_tensor(tmp[:], x_tiles[i][:, sl], ps_m[:],
                                        OP.subtract)
                out_sl_writer(i, tmp, ps_r)

        def layernorm(pool, x_tiles, nm, cols):
            """Full FM layernorm into fresh [128, cols] f32r tiles."""
            out = [pool.tile([128, cols], F32R, name=f"{nm}{i}", tag=f"{nm}{i}")
                   for i in range(NB)]
            with tc.tile_pool(name=f"{nm}_scr", bufs=1, side="left") as lp:
                for blk in range(cols // 512):
                    sl = slice(512 * blk, 512 * blk + 512)

                    def wr(i, tmp, ps_r, sl=sl):
                        nc.vector.tensor_tensor(out[i][:, sl], tmp[:], ps_r[:],
                                                OP.mult)
                    ln_block(lp, x_tiles, wr, sl)
            return out

        def gemm_ws(w_dram, Kf, Mf, rhs_tiles, cols, evict, nm, rhs_off=0):
            """out[m, col] = sum_k w[k, m] * rhs[k, col]; evict(mi, blk, psum)."""
            nkt = Kf // 128
            nblk = cols // 512
            mpg = max(1, 4 // nblk)   # m-tiles per group (<=4 psum accumulators)
            ntiles_m = Mf // 128
            for mg in range((ntiles_m + mpg - 1) // mpg):
                m4n = min(mpg, ntiles_m - mpg * mg)
                pt = [ptile("P0", name=f"{nm}_pa"), ptile("P1", name=f"{nm}_pb")]
                flat = [pt[j // 2][:, 512 * (j % 2):512 * (j % 2) + 512]
                        for j in range(4)]
                psums = [[flat[m4 * nblk + blk] for blk in range(nblk)]
                         for m4 in range(m4n)]
                cstart = 128 * mpg * mg
                cw = 128 * m4n
                for k in range(nkt):
                    ch = wp.tile([128, 512], F32R, name=f"{nm}_ch", tag="wchunk",
                                 bufs=8)
                    hw = cw // 2
                    nc.sync.dma_start(
                        out=ch[:, 0:hw],
                        in_=w_dram[128 * k:128 * k + 128, cstart:cstart + hw])
                    nc.gpsimd.dma_start(
                        out=ch[:, hw:cw],
                        in_=w_dram[128 * k:128 * k + 128, cstart + hw:cstart + cw])
                    for blk in range(nblk):
                        sl = slice(rhs_off + 512 * blk, rhs_off + 512 * blk + 512)
                        for m4 in range(m4n):
                            nc.tensor.matmul(
                                psums[m4][blk], ch[:, 128 * m4:128 * m4 + 128],
                                rhs_tiles[k][:, sl],
                                start=(k == 0), stop=(k == nkt - 1))
                for m4 in range(m4n):
                    for blk in range(nblk):
                        evict(mpg * mg + m4, blk, psums[m4][blk])

        def adapter(out_pool, x_tiles, ai, cols, extra_res, nm):
            """out = (extra_res or 0) + x + up(gelu(dn(LN(x)))). FM tiles out.

            LN runs block-wise: only one 512-col block of normalized data is
            alive at a time.
            """
            with ExitStack() as ies:
                ip = ies.enter_context(
                    tc.tile_pool(name=f"{nm}_scr", bufs=1, side="left"))
                nblk = cols // 512
                mT = ip.tile([BTL, cols], F32R, name=f"{nm}m", tag=f"{nm}m")
                dn_sb = []
                for k in range(NB):
                    w = ip.tile([128, BTL], F32R, name=f"{nm}dn{k}",
                                tag=f"{nm}dn{k}")
                    nc.sync.dma_start(out=w[:],
                                      in_=adn_d[ai][128 * k:128 * k + 128, :])
                    dn_sb.append(w)
                for blk in range(nblk):
                    sl = slice(512 * blk, 512 * blk + 512)
                    ps = ptile("P0" if blk % 2 == 0 else "P1", p=BTL, f=512,
                               name=f"{nm}_dps")

                    def wr(i, tmp, ps_r, ps=ps):
                        a = ip.tile([128, 512], F32R, name="ad_a", tag="ad_a",
                                    bufs=3)
                        nc.vector.tensor_tensor(a[:], tmp[:], ps_r[:],
                                                OP.mult)
                        nc.tensor.matmul(ps[:], dn_sb[i][:], a[:],
                                         start=(i == 0), stop=(i == NB - 1))
                    ln_block(ip, x_tiles, wr, sl)
                    nc.scalar.activation(mT[:, sl], ps[:], AF.Gelu,
                                         bias=adb_s[ai][:])
                up_sb = ip.tile([BTL, D], F32R, name=f"{nm}up", tag=f"{nm}up")
                nc.sync.dma_start(out=up_sb[:], in_=aup_d[ai][:])
                out = [out_pool.tile([128, cols], F32R, name=f"{nm}o{i}",
                                     tag=f"{nm}o{i}") for i in range(NB)]
                for m in range(NB):
                    for blk in range(nblk):
                        sl = slice(512 * blk, 512 * blk + 512)
                        ps = ptile("P0" if (m * nblk + blk) % 2 == 0 else "P1",
                                   f=512, name=f"{nm}_ups")
                        nc.tensor.matmul(ps[:], up_sb[:, 128 * m:128 * m + 128],
                                         mT[:, sl], start=True, stop=True)
                        tu = ip.tile([128, 512], F32, name="uptmp", tag="uptmp",
                                     bufs=2)
                        nc.scalar.activation(tu[:], ps[:], AF.Identity,
                                             bias=aub_s[ai][:, m:m + 1])
                        if extra_res is not None:
                            tu2 = ip.tile([128, 512], F32, name="uptmp2",
                                          tag="uptmp2", bufs=2)
                            nc.vector.tensor_tensor(tu2[:], tu[:],
                                                    x_tiles[m][:, sl], OP.add)
                            nc.vector.tensor_tensor(out[m][:, sl], tu2[:],
                                                    extra_res[m][:, sl], OP.add)
                        else:
                            nc.vector.tensor_tensor(out[m][:, sl], tu[:],
                                                    x_tiles[m][:, sl], OP.add)
            return out

        # ============================ block body ============================
        def half_block(xT_d_, is_dec):
            tagp = "d" if is_dec else "e"
            with ExitStack() as es:
                # right stack (long-lived): xown, proj
                xop = es.enter_context(
                    tc.tile_pool(name=f"{tagp}_xo", bufs=1, side="right"))
                xown = [xop.tile([128, Q], F32R, name=f"{tagp}xo{i}",
                                 tag=f"{tagp}xo{i}") for i in range(NB)]
                wop = es.enter_context(
                    tc.tile_pool(name=f"{tagp}_wo", bufs=1, side="right"))
                proj = [wop.tile([128, Q], F32R, name=f"{tagp}proj{i}",
                                 tag=f"{tagp}proj{i}") for i in range(NB)]

                # left stack: h1 outlives x/dad
                with ExitStack() as h1_es:
                    h1p = h1_es.enter_context(
                        tc.tile_pool(name=f"{tagp}_h1", bufs=1, side="left"))
                    if is_dec:
                        dad_es = ExitStack()
                        dadp = dad_es.enter_context(
                            tc.tile_pool(name="dadp", bufs=1, side="left"))
                    x_es = ExitStack()
                    xp = x_es.enter_context(
                        tc.tile_pool(name=f"{tagp}_x", bufs=1, side="left"))
                    xT = []
                    for i in range(NB):
                        t = xp.tile([128, T], F32R, name=f"{tagp}xT{i}",
                                    tag=f"{tagp}xT{i}")
                        nc.sync.dma_start(out=t[:],
                                          in_=xT_d_[128 * i:128 * i + 128, :])
                        xT.append(t)
                    for i in range(NB):
                        nc.vector.tensor_copy(xown[i][:], xT[i][:, 0:Q])
                    if is_dec:
                        mark(f"{tagp}:ad1")
                        dad = adapter(dadp, xT, 0, T, None, "ddad")
                        x_es.close()
                        mark(f"{tagp}:ln1")
                        h1 = layernorm(h1p, dad, f"{tagp}h1", T)
                        dad_es.close()
                    else:
                        mark(f"{tagp}:ln1")
                        h1 = layernorm(h1p, xT, f"{tagp}h1", T)
                        x_es.close()

                    # open attention-lifetime pools late (right side)
                    at_es = ExitStack()
                    atp = at_es.enter_context(
                        tc.tile_pool(name=f"{tagp}_attn", bufs=1, side="right"))
                    attnT = [atp.tile([128, Q], F32R, name=f"{tagp}atn{i}",
                                      tag=f"{tagp}atn{i}") for i in range(NB)]
                    if is_dec:
                        maskP = []
                        for i in range(2):
                            m = atp.tile([128, 2 * Q], F32, name=f"maskP{i}",
                                         tag=f"maskP{i}")
                            nc.sync.dma_start(
                                out=m[:], in_=maskP_d[128 * i:128 * i + 128, :])
                            maskP.append(m)
                    v_es = ExitStack()
                    vp_ = v_es.enter_context(
                        tc.tile_pool(name=f"{tagp}_vp", bufs=1, side="right"))
                    V = [vp_.tile([128, H * (DK + 1)], F32R, name=f"{tagp}V{i}",
                                  tag=f"{tagp}V{i}") for i in range(NB)]

                    # --- V (activation-stationary, wv streamed per half) ---
                    mark(f"{tagp}:V")
                    with tc.tile_pool(name=f"{tagp}_wv", bufs=1,
                                      side="left") as wvp:
                        for nblk in range(2):
                            wv_sb = []
                            for k in range(NB):
                                w = wvp.tile([128, 512], F32R,
                                             name=f"{tagp}wv{k}",
                                             tag=f"{tagp}wv{k}", bufs=1)
                                nc.sync.dma_start(
                                    out=w[:, 0:256],
                                    in_=wv_d[128 * k:128 * k + 128,
                                             512 * nblk:512 * nblk + 256])
                                nc.gpsimd.dma_start(
                                    out=w[:, 256:512],
                                    in_=wv_d[128 * k:128 * k + 128,
                                             512 * nblk + 256:512 * nblk + 512])
                                wv_sb.append(w)
                            for mr in range(NB):
                                j = (mr % 2) * 2 + nblk
                                vpt = ptile("P0" if j < 2 else "P1",
                                            name=f"{tagp}v_pt")
                                ps = vpt[:, 512 * (j % 2):512 * (j % 2) + 512]
                                for k in range(NB):
                                    nc.tensor.matmul(
                                        ps, h1[k][:, 128 * mr:128 * mr + 128],
                                        wv_sb[k][:],
                                        start=(k == 0), stop=(k == NB - 1))
                                dst = V[mr][:].rearrange("p (h e) -> p h e", h=H)
                                nc.vector.tensor_copy(
                                    dst[:, 8 * nblk:8 * nblk + 8, 0:DK],
                                    ps.rearrange("p (h e) -> p h e",
                                                 h=8, e=DK))
                        for mr in range(NB):
                            onesv = V[mr][:].rearrange(
                                "p (h e) -> p h e", h=H)[:, :, DK:DK + 1]
                            nc.vector.tensor_copy(
                                onesv[:], ones_col[:].to_broadcast((128, H, 1)))

                    # --- Q, K ---
                    qk_es = ExitStack()
                    qkp = qk_es.enter_context(
                        tc.tile_pool(name=f"{tagp}_qk", bufs=1, side="right"))
                    QT = [qkp.tile([128, Q], F32R, name=f"{tagp}QT{i}",
                                   tag=f"{tagp}QT{i}") for i in range(NB)]
                    KT = [qkp.tile([128, T], F32R, name=f"{tagp}KT{i}",
                                   tag=f"{tagp}KT{i}") for i in range(NB)]
                    mark(f"{tagp}:Q")

                    def ev_q(mi, blk, ps):
                        nc.scalar.activation(QT[mi][:], ps[:], AF.Identity,
                                             bias=bqkv_s[:, mi:mi + 1])
                    gemm_ws(wq_d, D, D, h1, 512, ev_q, f"{tagp}q")
                    mark(f"{tagp}:K")

                    def ev_k(mi, blk, ps):
                        nc.scalar.activation(
                            KT[mi][:, 512 * blk:512 * blk + 512], ps[:],
                            AF.Identity, bias=bqkv_s[:, NB + mi:NB + mi + 1])
                    gemm_ws(wk_d, D, D, h1, T, ev_k, f"{tagp}k")

                # --- attention (h1 freed) ---
                mark(f"{tagp}:attn")
                with tc.tile_pool(name=f"{tagp}_ascr", bufs=1, side="left") as ascp:
                    for tpair in range(NB):
                        attnP = ascp.tile([128, Q], F32, name="attnP",
                                          tag="attnP", bufs=2)
                        pden = ascp.tile([2, Q], F32, name="pden", tag="pden",
                                         bufs=2)
                        # AV psum: both heads side by side (2 banks)
                        ps_av = ptile("P2", p=DK + 1, name=f"{tagp}av_ps")
                        for hh in range(2):
                            h = 2 * tpair + hh
                            s = DK * hh
                            for ktp in range(NB // 2):
                                # scores for two k-tiles into one 2-bank psum
                                ps_sc = ptile("P0" if (ktp + 2 * hh) % 2 == 0
                                              else "P1", name=f"{tagp}sc_ps")
                                for u in range(2):
                                    kt = 2 * ktp + u
                                    nc.tensor.matmul(
                                        ps_sc[:, Q * u:Q * u + Q],
                                        KT[tpair][s:s + DK,
                                                  128 * kt:128 * kt + 128],
                                        QT[tpair][s:s + DK, :],
                                        start=True, stop=True)
                                em = ascp.tile([128, 2 * Q], F32R, name="expm",
                                               tag="expm", bufs=3)
                                if is_dec and ktp < 2:
                                    ex = ascp.tile([128, 2 * Q], F32, name="exps",
                                                   tag="exps", bufs=2)
                                    nc.scalar.activation(ex[:], ps_sc[:], AF.Exp)
                                    nc.vector.tensor_tensor(em[:], ex[:],
                                                            maskP[ktp][:],
                                                            OP.mult)
                                elif is_dec:
                                    ex = ascp.tile([128, 2 * Q], F32, name="exps",
                                                   tag="exps", bufs=2)
                                    nc.scalar.activation(ex[:], ps_sc[:], AF.Exp)
                                    nc.vector.tensor_scalar(
                                        em[:], ex[:], otherm_s[:, 0:1], None,
                                        OP.mult)
                                else:
                                    nc.scalar.activation(em[:], ps_sc[:], AF.Exp)
                                for u in range(2):
                                    kt = 2 * ktp + u
                                    nc.tensor.matmul(
                                        ps_av[:, Q * hh:Q * hh + Q],
                                        V[kt][:, 65 * h:65 * h + 65],
                                        em[:, Q * u:Q * u + Q],
                                        start=(kt == 0), stop=(kt == NB - 1))
                        stage = ascp.tile([DK + 1, 2 * Q], F32, name="avstage",
                                          tag="avstage", bufs=2)
                        nc.scalar.copy(stage[:], ps_av[:])
                        for hh in range(2):
                            s = DK * hh
                            nc.sync.dma_start(out=attnP[s:s + DK, :],
                                              in_=stage[0:DK, Q * hh:Q * hh + Q])
                            nc.sync.dma_start(
                                out=pden[hh:hh + 1, :],
                                in_=stage[DK:DK + 1, Q * hh:Q * hh + Q])
                        plnd = ascp.tile([2, Q], F32, name="plnd", tag="plnd")
                        nc.scalar.activation(plnd[:], pden[:], AF.Ln)
                        warm(plnd[:])
                        prec = ascp.tile([2, Q], F32R, name="prec", tag="prec")
                        nc.scalar.activation(prec[:], plnd[:], AF.Exp, scale=-1.0)
                        warm(plnd[:])
                        ps_b = ptile("P3", f=512, name=f"{tagp}bc_ps")
                        nc.tensor.matmul(ps_b[:], e2_s[:], prec[:],
                                         start=True, stop=True)
                        nc.vector.tensor_tensor(attnT[tpair][:], attnP[:],
                                                ps_b[:], OP.mult)
                qk_es.close()
                v_es.close()

                # --- Wo projection (+ residual & bo bias) ---
                if is_dec:
                    def ev_o(mi, blk, ps):
                        nc.scalar.activation(proj[mi][:], ps[:], AF.Identity,
                                             bias=bo_s[:, mi:mi + 1])
                else:
                    def ev_o(mi, blk, ps):
                        tz = wp.tile([128, 512], F32, name="wotmp", tag="wotmp",
                                     bufs=2)
                        nc.scalar.activation(tz[:], ps[:], AF.Identity,
                                             bias=bo_s[:, mi:mi + 1])
                        nc.vector.tensor_tensor(proj[mi][:], tz[:], xown[mi][:],
                                                OP.add)
                mark(f"{tagp}:wo")
                gemm_ws(wo_d, D, D, attnT, 512, ev_o, f"{tagp}o")
                at_es.close()

                # --- post-attention / FFN ---
                with ExitStack() as post_es:
                    if is_dec:
                        mark(f"{tagp}:ad2")
                        pop = post_es.enter_context(
                            tc.tile_pool(name=f"{tagp}_post", bufs=1,
                                         side="right"))
                        r1 = adapter(pop, proj, 1, Q, xown, "dr1")
                        ffn_res = r1
                    else:
                        ffn_res = proj

                    ffp = post_es.enter_context(
                        tc.tile_pool(name=f"{tagp}_ff", bufs=1, side="right"))
                    ffo = [ffp.tile([128, Q], F32R, name=f"{tagp}ffo{i}",
                                    tag=f"{tagp}ffo{i}") for i in range(NB)]

                    h2_es = ExitStack()
                    h2p = h2_es.enter_context(
                        tc.tile_pool(name=f"{tagp}_h2", bufs=1, side="left"))
                    if is_dec:
                        mark(f"{tagp}:ad3")
                        d2_es = ExitStack()
                        d2p = d2_es.enter_context(
                            tc.tile_pool(name="d2p", bufs=1, side="left"))
                        d2 = adapter(d2p, r1, 2, Q, None, "dd2")
                        mark(f"{tagp}:ln2")
                        h2 = layernorm(h2p, d2, f"{tagp}h2", Q)
                        d2_es.close()
                    else:
                        mark(f"{tagp}:ln2")
                        h2 = layernorm(h2p, ffn_res, f"{tagp}h2", Q)

                    yt_es = ExitStack()
                    ytp = yt_es.enter_context(
                        tc.tile_pool(name=f"{tagp}_yt", bufs=1, side="right"))
                    yT = [ytp.tile([128, Q], F32R, name=f"{tagp}yT{i}",
                                   tag=f"{tagp}yT{i}") for i in range(FF // 128)]

                    mark(f"{tagp}:w1")

                    def ev_y(mi, blk, ps):
                        nc.scalar.activation(yT[mi][:], ps[:], AF.Gelu,
                                             bias=b1_s[:, mi:mi + 1])
                    gemm_ws(w1_d, D, FF, h2, 512, ev_y, f"{tagp}w1")
                    mark(f"{tagp}:w2")
                    h2_es.close()

                    if is_dec:
                        def ev_z(mi, blk, ps):
                            nc.scalar.activation(ffo[mi][:], ps[:], AF.Identity,
                                                 bias=b2_s[:, mi:mi + 1])
                    else:
                        def ev_z(mi, blk, ps):
                            tz = wp.tile([128, 512], F32, name="ztmp",
                                         tag="wotmp", bufs=2)
                            nc.scalar.activation(tz[:], ps[:], AF.Identity,
                                                 bias=b2_s[:, mi:mi + 1])
                            nc.vector.tensor_tensor(ffo[mi][:], tz[:],
                                                    ffn_res[mi][:], OP.add)
                    gemm_ws(w2_d, FF, D, yT, 512, ev_z, f"{tagp}w2")
                    yt_es.close()

                    if is_dec:
                        mark(f"{tagp}:ad4")
                        doutp = post_es.enter_context(
                            tc.tile_pool(name=f"{tagp}_dout", bufs=1,
                                         side="right"))
                        dout = adapter(doutp, ffo, 3, Q, ffn_res, "ddo")
                        for i in range(NB):
                            nc.sync.dma_start(
                                out=decoutT_d[128 * i:128 * i + 128, :],
                                in_=dout[i][:].bitcast(F32))
                    else:
                        for i in range(NB):
                            nc.sync.dma_start(
                                out=encoutT_d[128 * i:128 * i + 128, :],
                                in_=ffo[i][:].bitcast(F32))

        half_block(xeT_d, is_dec=False)
        half_block(xdT_d, is_dec=True)

    nc.compile()
    return nc


def _get_program():
    global _COMPILED
    if _COMPILED is None:
        _COMPILED = _build()
    return _COMPILED


# ----------------------------------------------------------------------------
# host wrapper
# ----------------------------------------------------------------------------

def _np(x):
    return np.asarray(x, dtype=np.float32)


def make_in_maps(enc_x, dec_x, params, dec_causal_mask):
    p = params
    enc_x = _np(enc_x)
    dec_x = _np(dec_x)
    mask01 = np.asarray(dec_causal_mask)[0, 0].astype(np.float32)  # [T, T]

    def fold(g, b, w):
        w = _np(w)
        return (_np(g)[:, None] * w), (_np(b) @ w)

    wqkv = _np(p['enc_attn']['w_qkv'])
    g1, b1g = _np(p['enc_ln1_g']), _np(p['enc_ln1_b'])
    wq, bq = fold(g1, b1g, wqkv[:, 0:D])
    wq, bq = wq / 8.0, bq / 8.0
    wk, bk = fold(g1, b1g, wqkv[:, D:2 * D])
    wv, bv = fold(g1, b1g, wqkv[:, 2 * D:3 * D])
    wo = _np(p['enc_attn']['w_o'])
    bo = bv @ wo                      # v-bias folded through softmax (rows sum 1)
    g2, b2g = _np(p['enc_ln2_g']), _np(p['enc_ln2_b'])
    w1, b1f = fold(g2, b2g, _np(p['enc_ff']['w1']))
    b1f = b1f + _np(p['enc_ff']['b1'])
    w2 = _np(p['enc_ff']['w2'])
    b2f = _np(p['enc_ff']['b2'])

    def bias_cols(b, f):
        return np.ascontiguousarray(b.reshape(f // 128, 128).T).astype(np.float32)

    shared = {
        'wq': np.ascontiguousarray(wq), 'wk': np.ascontiguousarray(wk),
        'wv': np.ascontiguousarray(wv), 'wo': wo,
        'w1': np.ascontiguousarray(w1), 'w2': w2,
        'bqkv': np.concatenate([bias_cols(bq, D), bias_cols(bk, D)], axis=1),
        'bo': bias_cols(bo, D),
        'b1': bias_cols(b1f, FF), 'b2': bias_cols(b2f, D),
    }
    for i, nmi in enumerate(['adapt_pre_attn', 'adapt_post_attn',
                             'adapt_pre_ff', 'adapt_post_ff']):
        a = p[nmi]
        dw, db = fold(_np(a['ln_g']), _np(a['ln_b']), _np(a['down_w']))
        db = db + _np(a['down_b'])
        shared[f'adn{i}'] = np.ascontiguousarray(dw)
        shared[f'adb{i}'] = db.reshape(BTL, 1).astype(np.float32)
        shared[f'aup{i}'] = _np(a['up_w'])
        shared[f'aub{i}'] = bias_cols(_np(a['up_b']), D)

    in_maps = []
    for c in range(8):
        b, h = c // 2, c % 2
        own = np.arange(512 * h, 512 * h + 512)
        oth = np.arange(512 * (1 - h), 512 * (1 - h) + 512)
        perm = np.concatenate([own, oth])
        im = dict(shared)
        im['xeT'] = np.ascontiguousarray(enc_x[b][perm].T)     # [D, T] own-first
        im['xdT'] = np.ascontiguousarray(dec_x[b][perm].T)
        mo = mask01[own][:, own].T                                  # [k, q] own
        im['maskP'] = np.ascontiguousarray(np.concatenate(
            [np.concatenate([mo[256 * j:256 * j + 128, :],
                             mo[256 * j + 128:256 * j + 256, :]], axis=1)
             for j in range(2)], axis=0))
        other_blk = mask01[own][:, oth]
        ov = float(other_blk.flat[0])
        if not np.all(other_blk == ov):
            raise ValueError("decoder mask other-half is not constant per core; "
                             "unsupported mask structure")
        im['otherm'] = np.full((128, 1), ov, np.float32)
        in_maps.append(im)
    return in_maps


def assemble(results, B):
    enc_out = np.empty((B, T, D), np.float32)
    dec_out = np.empty((B, T, D), np.float32)
    for c in range(8):
        b, h = c // 2, c % 2
        own = slice(512 * h, 512 * h + 512)
        enc_out[b, own] = results[c]['encoutT'].T
        dec_out[b, own] = results[c]['decoutT'].T
    return enc_out, dec_out


def kernel(enc_x, dec_x, params, dec_causal_mask, cross_mask):
    in_maps = make_in_maps(enc_x, dec_x, params, dec_causal_mask)
    nc = _get_program()
    res = run_bass_kernel_spmd(nc, in_maps, core_ids=list(range(8)))
    return assemble(res.results, np.asarray(enc_x).shape[0])


# revision 18
# speedup vs baseline: 1.3807x; 1.0064x over previous
"""Trainium2 Bass kernel for nn_BridgeBlock (encoder/decoder bridge block).

Sharding: 8 cores = (batch 0..3) x (query-half 0..1). Each core computes its
512 query rows of both the encoder and decoder paths. K/V are computed
locally for all 1024 rows of the core's batch; per-core inputs are column-
rotated so the core's own 512 rows always sit in columns [0:512) (keeps the
program identical across cores / pure SPMD).

Layout: feature-major ("FM") — activations stored transposed [features, rows]
in SBUF as 8 tiles of [128, cols]. Weights-stationary matmuls produce FM
outputs; an activation-stationary matmul produces row-major V for attention.
Scores are computed transposed [k, q]; the softmax denominator comes free via
a ones-column appended to V; normalization is deferred to after A@V (recip =
exp(-ln), same ACT table set as the softmax exp). LayerNorm gain/bias, the
1/sqrt(dk) scale, and the V-bias (via softmax rows summing to 1) are folded
into weights/biases on the host. Decoder mask: own-block [512,512] tile mask
plus a per-core scalar for the other half (causal => other half is all-0 or
all-1; all-ones masks also supported). All matmuls run in float32r
(tf32-like, ~1.5e-4), everything else fp32.

SBUF: long-lived tensors allocate on the right side-stack, transients on the
left; PSUM uses 8 fixed bank tags.
"""
import numpy as np
from contextlib import ExitStack

import concourse.bass as bass
import concourse.tile as tile
import concourse.mybir as mybir
from concourse import bacc
from concourse.bass_utils import run_bass_kernel_spmd

F32 = mybir.dt.float32
F32R = mybir.dt.float32r
AF = mybir.ActivationFunctionType
OP = mybir.AluOpType

D = 1024      # d_model
FF = 4096     # d_ff
H = 16        # heads
DK = 64       # head dim
BTL = 64      # adapter bottleneck
Q = 512       # own query rows per core
T = 1024      # total rows per batch
NB = D // 128   # 8 feature tiles
EPS = 1e-5

_COMPILED = None
MARKS = []


def _build():
    nc = bacc.Bacc("TRN2", target_bir_lowering=False, debug=False)

    def din(name, shape, dt=F32R):
        return nc.dram_tensor(name, shape, dt, kind="ExternalInput").ap()

    xeT_d = din("xeT", [D, T])
    xdT_d = din("xdT", [D, T])
    maskP_d = din("maskP", [2 * 128, 2 * Q], F32)  # paired own-block mask
    otherm_d = din("otherm", [128, 1], F32)    # other-half mask scalar
    wq_d = din("wq", [D, D])
    wk_d = din("wk", [D, D])
    wv_d = din("wv", [D, D])
    wo_d = din("wo", [D, D])
    w1_d = din("w1", [D, FF])
    w2_d = din("w2", [FF, D])
    bqkv_d = din("bqkv", [128, NB * 2], F32)   # folded q/k biases, [128,8] each
    bo_d = din("bo", [128, NB], F32)           # bv @ wo fold (per out-feature)
    b1_d = din("b1", [128, FF // 128], F32)
    b2_d = din("b2", [128, NB], F32)
    adn_d = [din(f"adn{i}", [D, BTL]) for i in range(4)]
    aup_d = [din(f"aup{i}", [BTL, D]) for i in range(4)]
    adb_d = [din(f"adb{i}", [BTL, 1], F32) for i in range(4)]
    aub_d = [din(f"aub{i}", [128, NB], F32) for i in range(4)]

    encoutT_d = nc.dram_tensor("encoutT", [D, Q], F32, kind="ExternalOutput").ap()
    decoutT_d = nc.dram_tensor("decoutT", [D, Q], F32, kind="ExternalOutput").ap()

    # inline constants
    e2 = np.zeros((2, 128), np.float32)
    e2[0, 0:DK] = 1.0
    e2[1, DK:128] = 1.0
    e2_d = nc.inline_tensor(e2, name="e2c").ap()
    ones_row_d = nc.inline_tensor(np.ones((1, 128), np.float32), name="ones_row_c").ap()
    invd_row_d = nc.inline_tensor(np.full((1, 128), 1.0 / D, np.float32),
                                  name="invd_row_c").ap()
    ones_col_d = nc.inline_tensor(np.ones((128, 1), np.float32), name="ones_col_c").ap()

    MARKS.clear()

    def mark(label):
        MARKS.append((int(nc.get_next_instruction_name()[2:]), label))

    with tile.TileContext(nc) as tc, ExitStack() as top:
        cp = top.enter_context(tc.tile_pool(name="consts", bufs=1, side="left"))
        pp = top.enter_context(tc.tile_pool(name="psum", bufs=1, space="PSUM"))
        wp = top.enter_context(tc.tile_pool(name="wchunks", bufs=1, side="left"))

        def ptile(tag, p=128, f=1024, name="ps"):
            return pp.tile([p, f], F32, name=name, tag=tag)

        # ---- constants / biases ----
        e2_s = cp.tile([2, 128], F32R, name="e2_s")
        nc.sync.dma_start(out=e2_s[:], in_=e2_d[:].bitcast(F32R))
        ones_row = cp.tile([1, 128], F32R, name="ones_row")
        nc.sync.dma_start(out=ones_row[:], in_=ones_row_d[:].bitcast(F32R))
        invd_row = cp.tile([1, 128], F32R, name="invd_row")
        nc.sync.dma_start(out=invd_row[:], in_=invd_row_d[:].bitcast(F32R))
        ones_col = cp.tile([128, 1], F32R, name="ones_col")
        nc.sync.dma_start(out=ones_col[:], in_=ones_col_d[:].bitcast(F32R))
        bqkv_s = cp.tile([128, NB * 2], F32, name="bqkv_s")
        nc.sync.dma_start(out=bqkv_s[:], in_=bqkv_d[:])
        bo_s = cp.tile([128, NB], F32, name="bo_s")
        nc.sync.dma_start(out=bo_s[:], in_=bo_d[:])
        b1_s = cp.tile([128, FF // 128], F32, name="b1_s")
        nc.sync.dma_start(out=b1_s[:], in_=b1_d[:])
        b2_s = cp.tile([128, NB], F32, name="b2_s")
        nc.sync.dma_start(out=b2_s[:], in_=b2_d[:])
        otherm_s = cp.tile([128, 1], F32, name="otherm_s")
        nc.sync.dma_start(out=otherm_s[:], in_=otherm_d[:])
        eps_t = cp.tile([1, 1], F32, name="eps_t")
        nc.vector.memset(eps_t[:], EPS)
        adb_s, aub_s = [], []
        for i in range(4):
            a = cp.tile([BTL, 1], F32, name=f"adb_s{i}", tag=f"adb_s{i}")
            nc.sync.dma_start(out=a[:], in_=adb_d[i][:])
            adb_s.append(a)
            u = cp.tile([128, NB], F32, name=f"aub_s{i}", tag=f"aub_s{i}")
            nc.sync.dma_start(out=u[:], in_=aub_d[i][:])
            aub_s.append(u)

        # ---- helpers ----
        def warm(dep_ap):
            """Tiny dep-gated fp32 matmul to keep the PE HAM clock from idling."""
            psd = pp.tile([1, 64], F32, name="warm_ps", tag="P2")
            nc.tensor.matmul(psd[:], ones_row[0:1, 0:1].bitcast(F32),
                             dep_ap[0:1, 0:64], start=True, stop=True)

        def ln_block(lp, x_tiles, out_sl_writer, sl):
            """One 512-col LN block: stats + normalize.

            out = (x - mean_bcast) * rstd_bcast; mean bcast is issued early so
            the subtract half of the apply overlaps the rstd scalar chain.
            Uses only Copy/Ln/Exp ACT funcs (one table set).
            """
            st = ptile("P2", p=1, name="ln_stats")
            ps_s = st[0:1, 0:512]
            ps_q = st[0:1, 512:1024]
            for i in range(NB):
                sq = lp.tile([128, 512], F32R, name="ln_sq", tag="ln_sq", bufs=2)
                nc.vector.tensor_tensor(sq[:], x_tiles[i][:, sl],
                                        x_tiles[i][:, sl], OP.mult)
                nc.tensor.matmul(ps_s[:], ones_col[:], x_tiles[i][:, sl],
                                 start=(i == 0), stop=(i == NB - 1))
                nc.tensor.matmul(ps_q[:], ones_col[:], sq[:],
                                 start=(i == 0), stop=(i == NB - 1))
            sumr = lp.tile([1, 512], F32R, name="ln_sumr", tag="ln_sumr")
            nc.scalar.copy(sumr[:], ps_s[:])
            bc = ptile("P3", name="ln_bc")
            ps_m = bc[:, 0:512]                  # mean bcast, available early
            nc.tensor.matmul(ps_m[:], invd_row[:], sumr[:], start=True, stop=True)
            s2 = lp.tile([1, 512], F32, name="ln_s2", tag="ln_s2")
            nc.scalar.activation(s2[:], ps_s[:], AF.Square,
                                 scale=1.0 / float(np.sqrt(D)))   # sum^2/D
            nc.vector.tensor_tensor(s2[:], ps_q[:], s2[:], OP.subtract)
            warm(s2[:])
            nc.scalar.activation(s2[:], s2[:], AF.Ln, scale=1.0 / D,
                                 bias=eps_t[0:1, 0:1])
            rstd = lp.tile([1, 512], F32R, name="ln_rstd", tag="ln_rstd")
            nc.scalar.activation(rstd[:], s2[:], AF.Exp, scale=-0.5)
            ps_r = bc[:, 512:1024]
            nc.tensor.matmul(ps_r[:], ones_row[:], rstd[:], start=True, stop=True)
            for i in range(NB):
                tmp = lp.tile([128, 512], F32, name="ln_tmp", tag="ln_tmp", bufs=3)
                nc.vector.tensor_tensor(tmp[:], x_tiles[i][:, sl], ps_m[:],
                                        OP.subtract)
                out_sl_writer(i, tmp, ps_r)

        def layernorm(pool, x_tiles, nm, cols):
            """Full FM layernorm into fresh [128, cols] f32r tiles."""
            out = [pool.tile([128, cols], F32R, name=f"{nm}{i}", tag=f"{nm}{i}")
                   for i in range(NB)]
            with tc.tile_pool(name=f"{nm}_scr", bufs=1, side="left") as lp:
                for blk in range(cols // 512):
                    sl = slice(512 * blk, 512 * blk + 512)

                    def wr(i, tmp, ps_r, sl=sl):
                        nc.vector.tensor_tensor(out[i][:, sl], tmp[:], ps_r[:],
                                                OP.mult)
                    ln_block(lp, x_tiles, wr, sl)
            return out

        def gemm_ws(w_dram, Kf, Mf, rhs_tiles, cols, evict, nm, rhs_off=0):
            """out[m, col] = sum_k w[k, m] * rhs[k, col]; evict(mi, blk, psum)."""
            nkt = Kf // 128
            nblk = cols // 512
            mpg = max(1, 4 // nblk)   # m-tiles per group (<=4 psum accumulators)
            ntiles_m = Mf // 128
            for mg in range((ntiles_m + mpg - 1) // mpg):
                m4n = min(mpg, ntiles_m - mpg * mg)
                pt = [ptile("P0", name=f"{nm}_pa"), ptile("P1", name=f"{nm}_pb")]
                flat = [pt[j // 2][:, 512 * (j % 2):512 * (j % 2) + 512]
                        for j in range(4)]
                psums = [[flat[m4 * nblk + blk] for blk in range(nblk)]
                         for m4 in range(m4n)]
                cstart = 128 * mpg * mg
                cw = 128 * m4n
                for k in range(nkt):
                    ch = wp.tile([128, 512], F32R, name=f"{nm}_ch", tag="wchunk",
                                 bufs=12)
                    hw = cw // 2
                    nc.sync.dma_start(
                        out=ch[:, 0:hw],
                        in_=w_dram[128 * k:128 * k + 128, cstart:cstart + hw])
                    nc.gpsimd.dma_start(
                        out=ch[:, hw:cw],
                        in_=w_dram[128 * k:128 * k + 128, cstart + hw:cstart + cw])
                    for blk in range(nblk):
                        sl = slice(rhs_off + 512 * blk, rhs_off + 512 * blk + 512)
                        for m4 in range(m4n):
                            nc.tensor.matmul(
                                psums[m4][blk], ch[:, 128 * m4:128 * m4 + 128],
                                rhs_tiles[k][:, sl],
                                start=(k == 0), stop=(k == nkt - 1))
                for m4 in range(m4n):
                    for blk in range(nblk):
                        evict(mpg * mg + m4, blk, psums[m4][blk])

        def load_adw(pool, ai, nm):
            """Preload one adapter's down/up weights."""
            dn_sb = []
            for k in range(NB):
                w = pool.tile([128, BTL], F32R, name=f"{nm}dn{k}",
                              tag=f"{nm}dn{k}")
                nc.sync.dma_start(out=w[:],
                                  in_=adn_d[ai][128 * k:128 * k + 128, :])
                dn_sb.append(w)
            up_sb = pool.tile([BTL, D], F32R, name=f"{nm}up", tag=f"{nm}up")
            nc.gpsimd.dma_start(out=up_sb[:], in_=aup_d[ai][:])
            return dn_sb, up_sb

        def adapter(out_pool, x_tiles, ai, cols, extra_res, nm, adw=None):
            """out = (extra_res or 0) + x + up(gelu(dn(LN(x)))). FM tiles out.

            LN runs block-wise: only one 512-col block of normalized data is
            alive at a time.
            """
            with ExitStack() as ies:
                ip = ies.enter_context(
                    tc.tile_pool(name=f"{nm}_scr", bufs=1, side="left"))
                nblk = cols // 512
                mT = ip.tile([BTL, cols], F32R, name=f"{nm}m", tag=f"{nm}m")
                if adw is not None:
                    dn_sb, up_sb = adw
                else:
                    dn_sb, up_sb = load_adw(ip, ai, nm)
                for blk in range(nblk):
                    sl = slice(512 * blk, 512 * blk + 512)
                    ps = ptile("P0" if blk % 2 == 0 else "P1", p=BTL, f=512,
                               name=f"{nm}_dps")

                    def wr(i, tmp, ps_r, ps=ps):
                        a = ip.tile([128, 512], F32R, name="ad_a", tag="ad_a",
                                    bufs=3)
                        nc.vector.tensor_tensor(a[:], tmp[:], ps_r[:],
                                                OP.mult)
                        nc.tensor.matmul(ps[:], dn_sb[i][:], a[:],
                                         start=(i == 0), stop=(i == NB - 1))
                    ln_block(ip, x_tiles, wr, sl)
                    nc.scalar.activation(mT[:, sl], ps[:], AF.Gelu,
                                         bias=adb_s[ai][:])
                out = [out_pool.tile([128, cols], F32R, name=f"{nm}o{i}",
                                     tag=f"{nm}o{i}") for i in range(NB)]
                for m in range(NB):
                    for blk in range(nblk):
                        sl = slice(512 * blk, 512 * blk + 512)
                        ps = ptile("P0" if (m * nblk + blk) % 2 == 0 else "P1",
                                   f=512, name=f"{nm}_ups")
                        nc.tensor.matmul(ps[:], up_sb[:, 128 * m:128 * m + 128],
                                         mT[:, sl], start=True, stop=True)
                        tu = ip.tile([128, 512], F32, name="uptmp", tag="uptmp",
                                     bufs=2)
                        nc.scalar.activation(tu[:], ps[:], AF.Identity,
                                             bias=aub_s[ai][:, m:m + 1])
                        if extra_res is not None:
                            tu2 = ip.tile([128, 512], F32, name="uptmp2",
                                          tag="uptmp2", bufs=2)
                            nc.vector.tensor_tensor(tu2[:], tu[:],
                                                    x_tiles[m][:, sl], OP.add)
                            nc.vector.tensor_tensor(out[m][:, sl], tu2[:],
                                                    extra_res[m][:, sl], OP.add)
                        else:
                            nc.vector.tensor_tensor(out[m][:, sl], tu[:],
                                                    x_tiles[m][:, sl], OP.add)
            return out

        # ============================ block body ============================
        def half_block(xT_d_, is_dec):
            tagp = "d" if is_dec else "e"
            with ExitStack() as es:
                # right stack (long-lived): xown, proj
                xop = es.enter_context(
                    tc.tile_pool(name=f"{tagp}_xo", bufs=1, side="right"))
                xown = [xop.tile([128, Q], F32R, name=f"{tagp}xo{i}",
                                 tag=f"{tagp}xo{i}") for i in range(NB)]
                wop = es.enter_context(
                    tc.tile_pool(name=f"{tagp}_wo", bufs=1, side="right"))
                proj = [wop.tile([128, Q], F32R, name=f"{tagp}proj{i}",
                                 tag=f"{tagp}proj{i}") for i in range(NB)]

                # left stack: h1 outlives x/dad
                with ExitStack() as h1_es:
                    h1p = h1_es.enter_context(
                        tc.tile_pool(name=f"{tagp}_h1", bufs=1, side="left"))
                    if is_dec:
                        dad_es = ExitStack()
                        dadp = dad_es.enter_context(
                            tc.tile_pool(name="dadp", bufs=1, side="left"))
                    x_es = ExitStack()
                    xp = x_es.enter_context(
                        tc.tile_pool(name=f"{tagp}_x", bufs=1, side="left"))
                    xT = []
                    for i in range(NB):
                        t = xp.tile([128, T], F32R, name=f"{tagp}xT{i}",
                                    tag=f"{tagp}xT{i}")
                        nc.sync.dma_start(out=t[:],
                                          in_=xT_d_[128 * i:128 * i + 128, :])
                        xT.append(t)
                    for i in range(NB):
                        nc.vector.tensor_copy(xown[i][:], xT[i][:, 0:Q])
                    if is_dec:
                        mark(f"{tagp}:ad1")
                        dad = adapter(dadp, xT, 0, T, None, "ddad")
                        x_es.close()
                        mark(f"{tagp}:ln1")
                        h1 = layernorm(h1p, dad, f"{tagp}h1", T)
                        dad_es.close()
                    else:
                        mark(f"{tagp}:ln1")
                        h1 = layernorm(h1p, xT, f"{tagp}h1", T)
                        x_es.close()

                    # open attention-lifetime pools late (right side)
                    at_es = ExitStack()
                    atp = at_es.enter_context(
                        tc.tile_pool(name=f"{tagp}_attn", bufs=1, side="right"))
                    attnT = [atp.tile([128, Q], F32R, name=f"{tagp}atn{i}",
                                      tag=f"{tagp}atn{i}") for i in range(NB)]
                    if is_dec:
                        maskP = []
                        for i in range(2):
                            m = atp.tile([128, 2 * Q], F32, name=f"maskP{i}",
                                         tag=f"maskP{i}")
                            nc.sync.dma_start(
                                out=m[:], in_=maskP_d[128 * i:128 * i + 128, :])
                            maskP.append(m)
                    v_es = ExitStack()
                    vp_ = v_es.enter_context(
                        tc.tile_pool(name=f"{tagp}_vp", bufs=1, side="right"))
                    V = [vp_.tile([128, H * (DK + 1)], F32R, name=f"{tagp}V{i}",
                                  tag=f"{tagp}V{i}") for i in range(NB)]

                    # --- V (activation-stationary, wv streamed per half) ---
                    mark(f"{tagp}:V")
                    with tc.tile_pool(name=f"{tagp}_wv", bufs=1,
                                      side="left") as wvp:
                        for nblk in range(2):
                            wv_sb = []
                            for k in range(NB):
                                w = wvp.tile([128, 512], F32R,
                                             name=f"{tagp}wv{k}",
                                             tag=f"{tagp}wv{k}", bufs=1)
                                nc.sync.dma_start(
                                    out=w[:, 0:256],
                                    in_=wv_d[128 * k:128 * k + 128,
                                             512 * nblk:512 * nblk + 256])
                                nc.gpsimd.dma_start(
                                    out=w[:, 256:512],
                                    in_=wv_d[128 * k:128 * k + 128,
                                             512 * nblk + 256:512 * nblk + 512])
                                wv_sb.append(w)
                            for mr in range(NB):
                                j = (mr % 2) * 2 + nblk
                                vpt = ptile("P0" if j < 2 else "P1",
                                            name=f"{tagp}v_pt")
                                ps = vpt[:, 512 * (j % 2):512 * (j % 2) + 512]
                                for k in range(NB):
                                    nc.tensor.matmul(
                                        ps, h1[k][:, 128 * mr:128 * mr + 128],
                                        wv_sb[k][:],
                                        start=(k == 0), stop=(k == NB - 1))
                                dst = V[mr][:].rearrange("p (h e) -> p h e", h=H)
                                nc.vector.tensor_copy(
                                    dst[:, 8 * nblk:8 * nblk + 8, 0:DK],
                                    ps.rearrange("p (h e) -> p h e",
                                                 h=8, e=DK))
                        for mr in range(NB):
                            onesv = V[mr][:].rearrange(
                                "p (h e) -> p h e", h=H)[:, :, DK:DK + 1]
                            nc.vector.tensor_copy(
                                onesv[:], ones_col[:].to_broadcast((128, H, 1)))

                    # --- Q, K ---
                    qk_es = ExitStack()
                    qkp = qk_es.enter_context(
                        tc.tile_pool(name=f"{tagp}_qk", bufs=1, side="right"))
                    QT = [qkp.tile([128, Q], F32R, name=f"{tagp}QT{i}",
                                   tag=f"{tagp}QT{i}") for i in range(NB)]
                    KT = [qkp.tile([128, T], F32R, name=f"{tagp}KT{i}",
                                   tag=f"{tagp}KT{i}") for i in range(NB)]
                    mark(f"{tagp}:Q")

                    def ev_q(mi, blk, ps):
                        nc.scalar.activation(QT[mi][:], ps[:], AF.Identity,
                                             bias=bqkv_s[:, mi:mi + 1])
                    gemm_ws(wq_d, D, D, h1, 512, ev_q, f"{tagp}q")
                    mark(f"{tagp}:K")

                    def ev_k(mi, blk, ps):
                        nc.scalar.activation(
                            KT[mi][:, 512 * blk:512 * blk + 512], ps[:],
                            AF.Identity, bias=bqkv_s[:, NB + mi:NB + mi + 1])
                    gemm_ws(wk_d, D, D, h1, T, ev_k, f"{tagp}k")

                # --- attention (h1 freed) ---
                mark(f"{tagp}:attn")
                with tc.tile_pool(name=f"{tagp}_ascr", bufs=1, side="left") as ascp:
                    for tpair in range(NB):
                        attnP = ascp.tile([128, Q], F32, name="attnP",
                                          tag="attnP", bufs=2)
                        pden = ascp.tile([2, Q], F32, name="pden", tag="pden",
                                         bufs=2)
                        # AV psum: both heads side by side (2 banks)
                        ps_av = ptile("P2", p=DK + 1, name=f"{tagp}av_ps")
                        for hh in range(2):
                            h = 2 * tpair + hh
                            s = DK * hh
                            for ktp in range(NB // 2):
                                # scores for two k-tiles into one 2-bank psum
                                ps_sc = ptile("P0" if (ktp + 2 * hh) % 2 == 0
                                              else "P1", name=f"{tagp}sc_ps")
                                for u in range(2):
                                    kt = 2 * ktp + u
                                    nc.tensor.matmul(
                                        ps_sc[:, Q * u:Q * u + Q],
                                        KT[tpair][s:s + DK,
                                                  128 * kt:128 * kt + 128],
                                        QT[tpair][s:s + DK, :],
                                        start=True, stop=True)
                                em = ascp.tile([128, 2 * Q], F32R, name="expm",
                                               tag="expm", bufs=3)
                                if is_dec and ktp < 2:
                                    ex = ascp.tile([128, 2 * Q], F32, name="exps",
                                                   tag="exps", bufs=2)
                                    nc.scalar.activation(ex[:], ps_sc[:], AF.Exp)
                                    nc.vector.tensor_tensor(em[:], ex[:],
                                                            maskP[ktp][:],
                                                            OP.mult)
                                elif is_dec:
                                    ex = ascp.tile([128, 2 * Q], F32, name="exps",
                                                   tag="exps", bufs=2)
                                    nc.scalar.activation(ex[:], ps_sc[:], AF.Exp)
                                    nc.vector.tensor_scalar(
                                        em[:], ex[:], otherm_s[:, 0:1], None,
                                        OP.mult)
                                else:
                                    nc.scalar.activation(em[:], ps_sc[:], AF.Exp)
                                for u in range(2):
                                    kt = 2 * ktp + u
                                    nc.tensor.matmul(
                                        ps_av[:, Q * hh:Q * hh + Q],
                                        V[kt][:, 65 * h:65 * h + 65],
                                        em[:, Q * u:Q * u + Q],
                                        start=(kt == 0), stop=(kt == NB - 1))
                        stage = ascp.tile([DK + 1, 2 * Q], F32, name="avstage",
                                          tag="avstage", bufs=2)
                        nc.vector.tensor_copy(stage[:], ps_av[:])
                        for hh in range(2):
                            s = DK * hh
                            nc.sync.dma_start(out=attnP[s:s + DK, :],
                                              in_=stage[0:DK, Q * hh:Q * hh + Q])
                            nc.sync.dma_start(
                                out=pden[hh:hh + 1, :],
                                in_=stage[DK:DK + 1, Q * hh:Q * hh + Q])
                        plnd = ascp.tile([2, Q], F32, name="plnd", tag="plnd")
                        nc.scalar.activation(plnd[:], pden[:], AF.Ln)
                        warm(plnd[:])
                        prec = ascp.tile([2, Q], F32R, name="prec", tag="prec")
                        nc.scalar.activation(prec[:], plnd[:], AF.Exp, scale=-1.0)
                        warm(plnd[:])
                        ps_b = ptile("P3", f=512, name=f"{tagp}bc_ps")
                        nc.tensor.matmul(ps_b[:], e2_s[:], prec[:],
                                         start=True, stop=True)
                        nc.vector.tensor_tensor(attnT[tpair][:], attnP[:],
                                                ps_b[:], OP.mult)
                qk_es.close()
                v_es.close()

                # --- Wo projection (+ residual & bo bias) ---
                if is_dec:
                    def ev_o(mi, blk, ps):
                        nc.scalar.activation(proj[mi][:], ps[:], AF.Identity,
                                             bias=bo_s[:, mi:mi + 1])
                else:
                    def ev_o(mi, blk, ps):
                        tz = wp.tile([128, 512], F32, name="wotmp", tag="wotmp",
                                     bufs=2)
                        nc.scalar.activation(tz[:], ps[:], AF.Identity,
                                             bias=bo_s[:, mi:mi + 1])
                        nc.vector.tensor_tensor(proj[mi][:], tz[:], xown[mi][:],
                                                OP.add)
                mark(f"{tagp}:wo")
                gemm_ws(wo_d, D, D, attnT, 512, ev_o, f"{tagp}o")
                at_es.close()

                # --- post-attention / FFN ---
                with ExitStack() as post_es:
                    if is_dec:
                        adwp = post_es.enter_context(
                            tc.tile_pool(name="adw", bufs=1, side="left"))
                        adw2 = load_adw(adwp, 1, "dr1")
                        adw3 = load_adw(adwp, 2, "dd2")
                        adw4 = load_adw(adwp, 3, "ddo")
                        mark(f"{tagp}:ad2")
                        pop = post_es.enter_context(
                            tc.tile_pool(name=f"{tagp}_post", bufs=1,
                                         side="right"))
                        r1 = adapter(pop, proj, 1, Q, xown, "dr1", adw=adw2)
                        ffn_res = r1
                    else:
                        ffn_res = proj

                    ffp = post_es.enter_context(
                        tc.tile_pool(name=f"{tagp}_ff", bufs=1, side="right"))
                    ffo = [ffp.tile([128, Q], F32R, name=f"{tagp}ffo{i}",
                                    tag=f"{tagp}ffo{i}") for i in range(NB)]

                    h2_es = ExitStack()
                    h2p = h2_es.enter_context(
                        tc.tile_pool(name=f"{tagp}_h2", bufs=1, side="left"))
                    if is_dec:
                        mark(f"{tagp}:ad3")
                        d2_es = ExitStack()
                        d2p = d2_es.enter_context(
                            tc.tile_pool(name="d2p", bufs=1, side="left"))
                        d2 = adapter(d2p, r1, 2, Q, None, "dd2", adw=adw3)
                        mark(f"{tagp}:ln2")
                        h2 = layernorm(h2p, d2, f"{tagp}h2", Q)
                        d2_es.close()
                    else:
                        mark(f"{tagp}:ln2")
                        h2 = layernorm(h2p, ffn_res, f"{tagp}h2", Q)

                    yt_es = ExitStack()
                    ytp = yt_es.enter_context(
                        tc.tile_pool(name=f"{tagp}_yt", bufs=1, side="right"))
                    yT = [ytp.tile([128, Q], F32R, name=f"{tagp}yT{i}",
                                   tag=f"{tagp}yT{i}") for i in range(FF // 128)]

                    mark(f"{tagp}:w1")

                    def ev_y(mi, blk, ps):
                        nc.scalar.activation(yT[mi][:], ps[:], AF.Gelu,
                                             bias=b1_s[:, mi:mi + 1])
                    gemm_ws(w1_d, D, FF, h2, 512, ev_y, f"{tagp}w1")
                    mark(f"{tagp}:w2")
                    h2_es.close()

                    if is_dec:
                        def ev_z(mi, blk, ps):
                            nc.scalar.activation(ffo[mi][:], ps[:], AF.Identity,
                                                 bias=b2_s[:, mi:mi + 1])
                    else:
                        def ev_z(mi, blk, ps):
                            tz = wp.tile([128, 512], F32, name="ztmp",
                                         tag="wotmp", bufs=2)
                            nc.scalar.activation(tz[:], ps[:], AF.Identity,
                                                 bias=b2_s[:, mi:mi + 1])
                            nc.vector.tensor_tensor(ffo[mi][:], tz[:],
                                                    ffn_res[mi][:], OP.add)
                    gemm_ws(w2_d, FF, D, yT, 512, ev_z, f"{tagp}w2")
                    yt_es.close()

                    if is_dec:
                        mark(f"{tagp}:ad4")
                        doutp = post_es.enter_context(
                            tc.tile_pool(name=f"{tagp}_dout", bufs=1,
                                         side="right"))
                        dout = adapter(doutp, ffo, 3, Q, ffn_res, "ddo",
                                       adw=adw4)
                        for i in range(NB):
                            nc.sync.dma_start(
                                out=decoutT_d[128 * i:128 * i + 128, :],
                                in_=dout[i][:].bitcast(F32))
                    else:
                        for i in range(NB):
                            nc.sync.dma_start(
                                out=encoutT_d[128 * i:128 * i + 128, :],
                                in_=ffo[i][:].bitcast(F32))

        half_block(xeT_d, is_dec=False)
        half_block(xdT_d, is_dec=True)

    nc.compile()
    return nc


def _get_program():
    global _COMPILED
    if _COMPILED is None:
        _COMPILED = _build()
    return _COMPILED


# ----------------------------------------------------------------------------
# host wrapper
# ----------------------------------------------------------------------------

def _np(x):
    return np.asarray(x, dtype=np.float32)


def make_in_maps(enc_x, dec_x, params, dec_causal_mask):
    p = params
    enc_x = _np(enc_x)
    dec_x = _np(dec_x)
    mask01 = np.asarray(dec_causal_mask)[0, 0].astype(np.float32)  # [T, T]

    def fold(g, b, w):
        w = _np(w)
        return (_np(g)[:, None] * w), (_np(b) @ w)

    wqkv = _np(p['enc_attn']['w_qkv'])
    g1, b1g = _np(p['enc_ln1_g']), _np(p['enc_ln1_b'])
    wq, bq = fold(g1, b1g, wqkv[:, 0:D])
    wq, bq = wq / 8.0, bq / 8.0
    wk, bk = fold(g1, b1g, wqkv[:, D:2 * D])
    wv, bv = fold(g1, b1g, wqkv[:, 2 * D:3 * D])
    wo = _np(p['enc_attn']['w_o'])
    bo = bv @ wo                      # v-bias folded through softmax (rows sum 1)
    g2, b2g = _np(p['enc_ln2_g']), _np(p['enc_ln2_b'])
    w1, b1f = fold(g2, b2g, _np(p['enc_ff']['w1']))
    b1f = b1f + _np(p['enc_ff']['b1'])
    w2 = _np(p['enc_ff']['w2'])
    b2f = _np(p['enc_ff']['b2'])

    def bias_cols(b, f):
        return np.ascontiguousarray(b.reshape(f // 128, 128).T).astype(np.float32)

    shared = {
        'wq': np.ascontiguousarray(wq), 'wk': np.ascontiguousarray(wk),
        'wv': np.ascontiguousarray(wv), 'wo': wo,
        'w1': np.ascontiguousarray(w1), 'w2': w2,
        'bqkv': np.concatenate([bias_cols(bq, D), bias_cols(bk, D)], axis=1),
        'bo': bias_cols(bo, D),
        'b1': bias_cols(b1f, FF), 'b2': bias_cols(b2f, D),
    }
    for i, nmi in enumerate(['adapt_pre_attn', 'adapt_post_attn',
                             'adapt_pre_ff', 'adapt_post_ff']):
        a = p[nmi]
        dw, db = fold(_np(a['ln_g']), _np(a['ln_b']), _np(a['down_w']))
        db = db + _np(a['down_b'])
        shared[f'adn{i}'] = np.ascontiguousarray(dw)
        shared[f'adb{i}'] = db.reshape(BTL, 1).astype(np.float32)
        shared[f'aup{i}'] = _np(a['up_w'])
        shared[f'aub{i}'] = bias_cols(_np(a['up_b']), D)

    in_maps = []
    for c in range(8):
        b, h = c // 2, c % 2
        own = np.arange(512 * h, 512 * h + 512)
        oth = np.arange(512 * (1 - h), 512 * (1 - h) + 512)
        perm = np.concatenate([own, oth])
        im = dict(shared)
        im['xeT'] = np.ascontiguousarray(enc_x[b][perm].T)     # [D, T] own-first
        im['xdT'] = np.ascontiguousarray(dec_x[b][perm].T)
        mo = mask01[own][:, own].T                                  # [k, q] own
        im['maskP'] = np.ascontiguousarray(np.concatenate(
            [np.concatenate([mo[256 * j:256 * j + 128, :],
                             mo[256 * j + 128:256 * j + 256, :]], axis=1)
             for j in range(2)], axis=0))
        other_blk = mask01[own][:, oth]
        ov = float(other_blk.flat[0])
        if not np.all(other_blk == ov):
            raise ValueError("decoder mask other-half is not constant per core; "
                             "unsupported mask structure")
        im['otherm'] = np.full((128, 1), ov, np.float32)
        in_maps.append(im)
    return in_maps


def assemble(results, B):
    enc_out = np.empty((B, T, D), np.float32)
    dec_out = np.empty((B, T, D), np.float32)
    for c in range(8):
        b, h = c // 2, c % 2
        own = slice(512 * h, 512 * h + 512)
        enc_out[b, own] = results[c]['encoutT'].T
        dec_out[b, own] = results[c]['decoutT'].T
    return enc_out, dec_out


def kernel(enc_x, dec_x, params, dec_causal_mask, cross_mask):
    in_maps = make_in_maps(enc_x, dec_x, params, dec_causal_mask)
    nc = _get_program()
    res = run_bass_kernel_spmd(nc, in_maps, core_ids=list(range(8)))
    return assemble(res.results, np.asarray(enc_x).shape[0])
